# revision 1
# baseline (speedup 1.0000x reference)
"""GAT (2-layer, 8-head) fused Bass kernel for 8 trn2 NeuronCores.

Sharding: nodes (rows of x) split 512/core. Layer-1 h/s computed locally per
core, AllGather'd (h+ones in bf16, scores in fp32); each core computes its
512xN attention block for all 8 heads; layer-1 output xc (+ its layer-2
projection, ones and scores) AllGather'd again (fp32); each core computes its
512xN layer-2 attention block and the final log_softmax rows.

Key algebra: with s_i = h_i . a_src, d_j = h_j . a_dst,
  exp(leakyrelu(s_i + d_j)) = max(exp(s_i)exp(d_j), exp(.2 s_i)exp(.2 d_j))
and softmax over j is invariant to any per-i scale, so the attention
numerator can be taken as P[i,j] = max(b_j, w_i * dd_j) with
  b_j = exp(d_j), w_i = exp(-0.8 s_i), dd_j = exp(0.2 d_j).
One DVE/GPSIMD tensor_scalar (mult, max) per [128,512] tile; exp only on
vectors. elu(x) = max(x, min(exp(x)-1, 0)); log_softmax via Exp(accum_out)+Ln.
Matmuls run as float32r (1 cyc/row) or bf16; fp32 would be 4 cyc/row.
"""

import numpy as np

N, NFEAT, NHID, NCLASS, NHEADS = 4096, 512, 64, 16, 8
NC = 8                      # cores
NQ = N // NC                # 512 own nodes per core
QT = NQ // 128              # 4 query tiles per core
JT = N // 128               # 32 key tiles
ALPHA = 0.2
HW = NHID * NHEADS          # 512
HXC = NHEADS * (NHID + 1)   # 520: per-head 64 h cols + ones col (bf16 AG1)
AGC2 = 18                   # AG2: 16 outh + 1 ones + 1 sdst2

_CACHE = {}


def _build_nc(no_cc=False, no_l1=False):
    import concourse.bass as bass
    import concourse.bacc as bacc
    import concourse.mybir as mybir
    import concourse.tile as tile
    from concourse.masks import make_identity

    fp32 = mybir.dt.float32
    fp32r = mybir.dt.float32r
    bf16 = mybir.dt.bfloat16
    AX = mybir.AxisListType.X
    OP = mybir.AluOpType
    AF = mybir.ActivationFunctionType

    nc = bacc.Bacc()
    xT = nc.declare_dram_parameter("xT", [NFEAT, NQ], fp32, isOutput=False)
    Whr = nc.declare_dram_parameter("Whr", [NFEAT, HW], fp32, isOutput=False)
    Asd = nc.declare_dram_parameter("Asd", [NFEAT, 16], fp32, isOutput=False)
    Wo = nc.declare_dram_parameter("Wo", [HW, NCLASS], fp32, isOutput=False)
    aod = nc.declare_dram_parameter("aod", [2, NCLASS], fp32, isOutput=False)
    out = nc.declare_dram_parameter("out", [NQ, NCLASS], fp32, isOutput=True)

    with tile.TileContext(nc) as tc:
        with (
            tc.tile_pool(name="const", bufs=1) as constp,
            tc.tile_pool(name="big", bufs=1) as bigp,
            tc.tile_pool(name="work", bufs=3) as workp,
            tc.tile_pool(name="pp", bufs=8) as ppool,
            tc.tile_pool(name="ps_acc", bufs=3, space="PSUM") as ps_acc,
            tc.tile_pool(name="ps_t", bufs=4, space="PSUM") as ps_t,
            tc.tile_pool(name="dram", bufs=1, space="DRAM") as dramp,
        ):
            v, sc, g, te, dma = nc.vector, nc.scalar, nc.gpsimd, nc.tensor, nc.sync

            ident = constp.tile([128, 128], fp32, tag="ident")
            make_identity(nc, ident[:])
            # sel[k, h*128+m] = 1 iff k == h: one-hot row selector for
            # partition-broadcast matmuls (out = sel_h.T @ rows)
            self_f = constp.tile([8, 8 * 128], fp32, tag="self_f")
            g.memset(self_f[:], 0.0)
            g.affine_select(
                out=self_f[:].rearrange("k (h m) -> k h m", m=128),
                in_=self_f[:].rearrange("k (h m) -> k h m", m=128),
                compare_op=mybir.AluOpType.not_equal,
                fill=1.0, base=0, channel_multiplier=1,
                pattern=[[-1, 8], [0, 128]])
            sel = constp.tile([8, 8 * 128], fp32r, tag="sel")
            v.tensor_copy(sel[:], self_f[:])

            # ---- A. load params (fp32 load + fp32r cast for PE use) ----
            xT_sb, whr_sb, asd_sb, wo_sb = [], [], [], []
            for k in range(4):
                tf = workp.tile([128, NQ], fp32, tag="pload", name=f"xTf{k}")
                dma.dma_start(tf[:], xT[k * 128:(k + 1) * 128, :])
                t = constp.tile([128, NQ], fp32r, tag=f"xT{k}", name=f"xT{k}")
                v.tensor_copy(t[:], tf[:])
                xT_sb.append(t)
                tf = workp.tile([128, HW], fp32, tag="pload", name=f"whrf{k}")
                dma.dma_start(tf[:], Whr[k * 128:(k + 1) * 128, :])
                t = constp.tile([128, HW], fp32r, tag=f"whr{k}", name=f"whr{k}")
                v.tensor_copy(t[:], tf[:])
                whr_sb.append(t)
                tf = workp.tile([128, 16], fp32, tag="pload16", name=f"asdf{k}")
                dma.dma_start(tf[:], Asd[k * 128:(k + 1) * 128, :])
                t = constp.tile([128, 16], fp32r, tag=f"asd{k}", name=f"asd{k}")
                v.tensor_copy(t[:], tf[:])
                asd_sb.append(t)
                t = constp.tile([128, 16], fp32, tag=f"wo{k}", name=f"wo{k}")
                dma.dma_start(t[:], Wo[k * 128:(k + 1) * 128, :])
                wo_sb.append(t)
            aos_b = constp.tile([128, 16], fp32, tag="aos_b")
            dma.dma_start(aos_b[:], aod[0:1, :].to_broadcast((128, 16)))
            aod_b = constp.tile([128, 16], fp32, tag="aod_b")
            dma.dma_start(aod_b[:], aod[1:2, :].to_broadcast((128, 16)))

            ag1h_in = dramp.tile([NQ, HXC], bf16, tag="ag1h_in")
            ag1h_out = dramp.tile([N, HXC], bf16, tag="ag1h_out",
                                  addr_space="Local" if no_cc else "Shared")
            ag1s_in = dramp.tile([NQ, 16], fp32, tag="ag1s_in")
            ag1s_out = dramp.tile([N, 16], fp32, tag="ag1s_out",
                                  addr_space="Local" if no_cc else "Shared")
            ag2_in = dramp.tile([NQ, AGC2], fp32, tag="ag2_in")
            ag2_out = dramp.tile([N, AGC2], fp32, tag="ag2_out",
                                 addr_space="Local" if no_cc else "Shared")

            # ---- B. h_ownT (feat-major) ----
            hT_sb = []
            for f in range(4):
                ps = ps_acc.tile([128, NQ], fp32, tag="acc")
                for k in range(4):
                    te.matmul(ps[:], whr_sb[k][:, f * 128:(f + 1) * 128],
                              xT_sb[k][:], start=(k == 0), stop=(k == 3))
                t = constp.tile([128, NQ], fp32r, tag=f"hT{f}", name=f"hT{f}")
                (v.tensor_copy if f % 2 else sc.copy)(t[:], ps[:])
                hT_sb.append(t)

            # ---- D. s_own [16, NQ] rows 0:8 src, 8:16 dst ----
            s_ps = ps_acc.tile([16, NQ], fp32, tag="acc")
            for k in range(4):
                te.matmul(s_ps[:], asd_sb[k][:], hT_sb[k][:],
                          start=(k == 0), stop=(k == 3))
            s_sb = constp.tile([16, NQ], fp32, tag="s_sb")
            v.tensor_copy(s_sb[:], s_ps[:])

            # ---- F. w panel: exp(-0.8 * s_src), bcast via K=1 matmul ----
            w_sb = constp.tile([NHEADS, NQ], fp32r, tag="w_sb")
            sc.activation(w_sb[:], s_sb[0:NHEADS, :], AF.Exp, scale=-0.8)
            wb = []
            for h in range(NHEADS):
                bp = ps_t.tile([128, NQ], fp32, tag="bigtp", bufs=1)
                te.matmul(bp[:], sel[:, h * 128:(h + 1) * 128],
                          w_sb[:], start=True, stop=True)
                t = constp.tile([128, NQ], bf16, tag=f"wb{h}", name=f"wb{h}")
                (v.tensor_copy if h % 2 else sc.copy)(t[:], bp[:])
                wb.append(t)

            # ---- C/E. h_own + stage AG1 (h bf16 + s fp32) ----
            for qt in range(QT):
                ps = ps_acc.tile([128, HW], fp32, tag="acc")
                for k in range(4):
                    te.matmul(ps[:], xT_sb[k][:, qt * 128:(qt + 1) * 128],
                              whr_sb[k][:], start=(k == 0), stop=(k == 3))
                stg = workp.tile([128, HXC], bf16, tag="stage")
                sc.copy(stg[:].rearrange("p (h c) -> p h c", c=65)[:, :, 0:64],
                        ps[:].rearrange("p (h c) -> p h c", c=64))
                g.memset(
                    stg[:].rearrange("p (h c) -> p h c", c=65)[:, :, 64:65], 1.0)
                dma.dma_start(ag1h_in[qt * 128:(qt + 1) * 128, :], stg[:])
                tp = ps_t.tile([128, 16], fp32, tag="tp")
                te.transpose(tp[:], s_sb[:, qt * 128:(qt + 1) * 128],
                             ident[0:16, 0:16])
                stgs = workp.tile([128, 16], fp32, tag="stgs")
                v.tensor_copy(stgs[:], tp[:])
                dma.dma_start(ag1s_in[qt * 128:(qt + 1) * 128, :], stgs[:])

            # ---- G. AllGather 1 (both buffers in one op) ----
            if no_cc:
                for r in range(NC):
                    dma.dma_start(ag1h_out[r * NQ:(r + 1) * NQ, :], ag1h_in[:])
                    dma.dma_start(ag1s_out[r * NQ:(r + 1) * NQ, :], ag1s_in[:])
            else:
                g.collective_compute(
                    "AllGather", OP.bypass,
                    ins=[ag1s_in.opt()], outs=[ag1s_out.opt()],
                    replica_groups=[list(range(NC))],
                )
                g.collective_compute(
                    "AllGather", OP.bypass,
                    ins=[ag1h_in.opt()], outs=[ag1h_out.opt()],
                    replica_groups=[list(range(NC))],
                )

            # ---- H. key-side score panels (bf16) ----
            sd_pan = constp.tile([128, JT * NHEADS], fp32, tag="sd_pan")
            dma.dma_start(
                sd_pan[:].rearrange("p (t h) -> p t h", h=NHEADS),
                ag1s_out[:, 8:16].rearrange("(t p) h -> p t h", p=128))
            b_all = constp.tile([128, JT * NHEADS], fp32, tag="b_all")
            sc.activation(b_all[:], sd_pan[:], AF.Exp)
            d_all = constp.tile([128, JT * NHEADS], fp32, tag="d_all")
            sc.activation(d_all[:], sd_pan[:], AF.Exp, scale=ALPHA)

            # ---- I. hx tiles (persistent keys, bf16) ----
            hx = []
            for jt in range(JT):
                t = bigp.tile([128, HXC], bf16, tag=f"hx{jt}", name=f"hx{jt}")
                dma.dma_start(t[:], ag1h_out[jt * 128:(jt + 1) * 128, :])
                hx.append(t)

            # ---- J/K. layer-1 attention ----
            xr = [bigp.tile([128, HW], fp32, tag=f"xr{qt}", name=f"xr{qt}")
                  for qt in range(QT)]
            xc_sb = [bigp.tile([128, HW], fp32, tag=f"xc{qt}", name=f"xc{qt}")
                     for qt in range(QT)]
            xcT_sb = [constp.tile([128, NQ], fp32, tag=f"xcT{f}", name=f"xcT{f}")
                      for f in range(4)]

            def elu_block(qt, fb):
                # elu on xr cols of head pair fb -> xc_sb, then transpose
                # into xcT_sb[fb] (overlaps with later heads' attention)
                c0, c1 = fb * 128, (fb + 1) * 128
                ex = workp.tile([128, 128], fp32, tag="ex")
                sc.activation(ex[:], xr[qt][:, c0:c1], AF.Exp)
                v.tensor_scalar(ex[:], ex[:], 1.0, 0.0, OP.subtract, OP.min)
                v.tensor_tensor(xc_sb[qt][:, c0:c1], xr[qt][:, c0:c1], ex[:],
                                OP.max)
                tp = ps_t.tile([128, 128], fp32, tag="tp")
                te.transpose(tp[:], xc_sb[qt][:, c0:c1], ident[:])
                eng_copy = sc.copy if fb % 2 else v.tensor_copy
                eng_copy(xcT_sb[fb][:, qt * 128:(qt + 1) * 128], tp[:])

            for h in range(NHEADS if not no_l1 else 0):
                acc = ps_acc.tile([65, NQ], fp32, tag="acc")
                for jt in range(JT):
                    pt = ppool.tile([128, NQ], bf16, tag="pt")
                    eng = g if (jt % 6 == 5) else v
                    eng.tensor_scalar(
                        pt[:], wb[h][:],
                        d_all[:, jt * NHEADS + h:jt * NHEADS + h + 1],
                        b_all[:, jt * NHEADS + h:jt * NHEADS + h + 1],
                        OP.mult, OP.max)
                    te.matmul(acc[:], hx[jt][:, h * 65:(h + 1) * 65], pt[:],
                              start=(jt == 0), stop=(jt == JT - 1))
                fT = workp.tile([65, NQ], fp32, tag="fT")
                sc.copy(fT[:], acc[:])
                den = workp.tile([128, QT], fp32, tag="den")
                tps = []
                for qt in range(QT):
                    tp = ps_t.tile([128, 65], fp32, tag="tp", name=f"tp{qt}")
                    te.transpose(tp[:], fT[:, qt * 128:(qt + 1) * 128],
                                 ident[0:65, 0:65])
                    sc.copy(den[:, qt:qt + 1], tp[:, 64:65])
                    tps.append(tp)
                r = workp.tile([128, QT], fp32, tag="recip")
                v.reciprocal(r[:], den[:])
                for qt in range(QT):
                    v.tensor_scalar(xr[qt][:, h * 64:(h + 1) * 64],
                                    tps[qt][:, 0:64], r[:, qt:qt + 1], None,
                                    OP.mult)
                if h % 2 == 1:
                    for qt in range(QT):
                        elu_block(qt, h // 2)

            # ---- K2/L fallback for no_l1 timing variant ----
            w2tmp = constp.tile([128, QT], fp32, tag="w2tmp")
            if no_l1:
                for qt in range(QT):
                    g.memset(xr[qt][:], 0.5)
                for qt in range(QT):
                    for fb in range(4):
                        elu_block(qt, fb)
            stg2s = [bigp.tile([128, AGC2], fp32, tag=f"stage2_{qt}",
                               name=f"stage2_{qt}") for qt in range(QT)]

            # ---- M. outh_own; scores; stage AG2 ----
            for qt in range(QT):
                ps = ps_t.tile([128, 16], fp32, tag="tp")
                for k in range(4):
                    te.matmul(ps[:], xcT_sb[k][:, qt * 128:(qt + 1) * 128],
                              wo_sb[k][:], start=(k == 0), stop=(k == 3))
                stg = stg2s[qt]
                v.tensor_copy(stg[:, 0:16], ps[:])
                g.memset(stg[:, 16:17], 1.0)
                tmp = workp.tile([128, 16], fp32, tag="sdtmp")
                v.tensor_tensor(tmp[:], ps[:], aod_b[:], OP.mult)
                v.tensor_reduce(stg[:, 17:18], tmp[:], AX, OP.add)
                v.tensor_tensor(tmp[:], ps[:], aos_b[:], OP.mult)
                v.tensor_reduce(w2tmp[:, qt:qt + 1], tmp[:], AX, OP.add)
                dma.dma_start(ag2_in[qt * 128:(qt + 1) * 128, :], stg[:])

            # ---- N. w2 bcast: [128,QT] -> row [1,NQ] -> bcast matmul ----
            w2e = constp.tile([128, QT], fp32, tag="w2e")
            sc.activation(w2e[:], w2tmp[:], AF.Exp, scale=-0.8)
            w2tp = ps_t.tile([QT, 128], fp32, tag="tp")
            te.transpose(w2tp[:], w2e[:], ident[:])
            w2tps = constp.tile([QT, 128], fp32r, tag="w2tps")
            v.tensor_copy(w2tps[:], w2tp[:])
            w2b = constp.tile([128, NQ], bf16, tag="w2b")
            for qt in range(QT):
                w2ps = ps_t.tile([128, 128], fp32, tag="tp")
                te.matmul(w2ps[:], sel[0:QT, qt * 128:(qt + 1) * 128],
                          w2tps[:], start=True, stop=True)
                sc.copy(w2b[:, qt * 128:(qt + 1) * 128], w2ps[:])

            # ---- O. AllGather 2 ----
            if no_cc:
                for r in range(NC):
                    dma.dma_start(ag2_out[r * NQ:(r + 1) * NQ, :], ag2_in[:])
            else:
                g.collective_compute(
                    "AllGather", OP.bypass,
                    ins=[ag2_in.opt()], outs=[ag2_out.opt()],
                    replica_groups=[list(range(NC))],
                )

            # ---- P. layer-2 panels ----
            hx2f = constp.tile([128, JT * 17], fp32, tag="hx2f")
            dma.dma_start(
                hx2f[:].rearrange("p (t c) -> p t c", c=17),
                ag2_out[:, 0:17].rearrange("(t p) c -> p t c", p=128))
            hx2 = constp.tile([128, JT * 17], bf16, tag="hx2")
            sc.copy(hx2[:], hx2f[:])
            sd2 = constp.tile([128, JT], fp32, tag="sd2")
            dma.dma_start(
                sd2[:].rearrange("p (t c) -> p t c", c=1),
                ag2_out[:, 17:18].rearrange("(t p) c -> p t c", p=128))
            b2 = constp.tile([128, JT], fp32, tag="b2")
            sc.activation(b2[:], sd2[:], AF.Exp)
            d2 = constp.tile([128, JT], fp32, tag="d2")
            sc.activation(d2[:], sd2[:], AF.Exp, scale=ALPHA)

            # ---- Q. layer-2 attention ----
            acc2 = ps_acc.tile([17, NQ], fp32, tag="acc")
            for jt in range(JT):
                pt = ppool.tile([128, NQ], bf16, tag="pt")
                eng = g if (jt % 6 == 5) else v
                eng.tensor_scalar(pt[:], w2b[:],
                                  d2[:, jt:jt + 1], b2[:, jt:jt + 1],
                                  OP.mult, OP.max)
                te.matmul(acc2[:], hx2[:, jt * 17:(jt + 1) * 17], pt[:],
                          start=(jt == 0), stop=(jt == JT - 1))
            f2 = workp.tile([17, NQ], fp32, tag="f2")
            sc.copy(f2[:], acc2[:])

            # ---- R. normalize, elu, log_softmax, store (ACT batched) ----
            den2 = workp.tile([128, QT], fp32, tag="den")
            t2s = []
            for qt in range(QT):
                tp = ps_t.tile([128, 17], fp32, tag="tp", name=f"t2_{qt}")
                te.transpose(tp[:], f2[:, qt * 128:(qt + 1) * 128],
                             ident[0:17, 0:17])
                sc.copy(den2[:, qt:qt + 1], tp[:, 16:17])
                t2s.append(tp)
            r2 = workp.tile([128, QT], fp32, tag="recip")
            v.reciprocal(r2[:], den2[:])
            os_, eos, elus, ses = [], [], [], []
            for qt in range(QT):
                o = workp.tile([128, NCLASS], fp32, tag=f"o{qt}", name=f"o{qt}")
                v.tensor_scalar(o[:], t2s[qt][:, 0:16], r2[:, qt:qt + 1], None,
                                OP.mult)
                os_.append(o)
            for qt in range(QT):
                eo = workp.tile([128, NCLASS], fp32, tag=f"eo{qt}",
                                name=f"eo{qt}")
                sc.activation(eo[:], os_[qt][:], AF.Exp)
                eos.append(eo)
            for qt in range(QT):
                v.tensor_scalar(eos[qt][:], eos[qt][:], 1.0, 0.0,
                                OP.subtract, OP.min)
                elu = workp.tile([128, NCLASS], fp32, tag=f"elu{qt}",
                                 name=f"elu{qt}")
                v.tensor_tensor(elu[:], os_[qt][:], eos[qt][:], OP.max)
                elus.append(elu)
            for qt in range(QT):
                se = workp.tile([128, 1], fp32, tag=f"se{qt}", name=f"se{qt}")
                e2 = workp.tile([128, NCLASS], fp32, tag="e2")
                sc.activation(e2[:], elus[qt][:], AF.Exp, accum_out=se[:])
                ses.append(se)
            lses = []
            for qt in range(QT):
                lse = workp.tile([128, 1], fp32, tag=f"lse{qt}",
                                 name=f"lse{qt}")
                sc.activation(lse[:], ses[qt][:], AF.Ln)
                lses.append(lse)
            for qt in range(QT):
                fin = workp.tile([128, NCLASS], fp32, tag="fin")
                v.tensor_scalar(fin[:], elus[qt][:], lses[qt][:], None,
                                OP.subtract)
                dma.dma_start(out[qt * 128:(qt + 1) * 128, :], fin[:])

    nc.finalize()
    return nc


def _get_compiled(no_cc=False, no_l1=False):
    key = ("nc", no_cc, no_l1)
    if key not in _CACHE:
        _CACHE[key] = _build_nc(no_cc=no_cc, no_l1=no_l1)
    return _CACHE[key]


def kernel(x, Wh, ah, Wo, ao):
    from concourse.bass_utils import run_bass_kernel_spmd

    nc = _get_compiled()
    x = np.asarray(x, np.float32)
    Wh = np.asarray(Wh, np.float32)
    ah = np.asarray(ah, np.float32)
    Wo = np.asarray(Wo, np.float32)
    ao = np.asarray(ao, np.float32)

    # host-side relayouts (no math): head-major weight matrix, block-diag
    # score matrix, split ao
    Whr = np.ascontiguousarray(
        Wh.transpose(1, 0, 2).reshape(NFEAT, HW))          # [512, 512]
    Asd = np.zeros((NFEAT, 16), np.float32)
    for h in range(NHEADS):
        Asd[h * NHID:(h + 1) * NHID, h] = ah[h, :NHID]      # src
        Asd[h * NHID:(h + 1) * NHID, 8 + h] = ah[h, NHID:]  # dst
    aod = np.stack([ao[:NCLASS], ao[NCLASS:]])              # [2, 16]

    in_maps = []
    for i in range(NC):
        in_maps.append({
            "xT": np.ascontiguousarray(x[i * NQ:(i + 1) * NQ].T),
            "Whr": Whr, "Asd": Asd,
            "Wo": np.ascontiguousarray(Wo), "aod": aod,
        })
    res = run_bass_kernel_spmd(nc, in_maps, list(range(NC)))
    return np.concatenate([res.results[i]["out"] for i in range(NC)], 0)



# revision 5
# speedup vs baseline: 1.3078x; 1.3078x over previous
"""GAT (2-layer, 8-head) fused Bass kernel for 8 trn2 NeuronCores.

Sharding: nodes (rows of x) split 512/core. Per core: h computed key-major
with fused score columns; h (bf16) + s_dst (fp32) AllGather'd; each core
computes its 512xN attention block for all 8 heads; layer-1 output projected
and AllGather'd (18 fp32 cols); each core computes its 512xN layer-2 block
and the final log_softmax rows.

Key algebra: with s_i = h_i . a_src, d_j = h_j . a_dst,
  exp(leakyrelu(s_i + d_j)) = max(exp(s_i)exp(d_j), exp(.2 s_i)exp(.2 d_j))
and softmax over j is invariant to any per-i scale, so the attention
numerator is P[j,i] = max(b_j, w_i * dd_j) with b_j = exp(d_j),
w_i = exp(-0.8 s_i), dd_j = exp(0.2 d_j).

P tiles [128 keys, 512 queries] are produced on three engines:
  DVE/Pool: tensor_scalar (mult, max) -> P
  ACT:      relu(dd_j * w_i - b_j) = P - b_j, single activation op; the
            missing rank-1 term hb[c] = sum_j b_j hx[j,c] over ACT-tiles is
            added back into the PSUM accumulation via two tiny matmuls.
Attention matmuls run with the P chunk [128k x 128q] as the *stationary*
operand and the per-head hx block [128, 65] (64 h cols + ones) as the
*moving* operand: 65 columns/matmul instead of 512 -> ~2x less PE time,
and the output lands query-major so normalize/elu/log_softmax use cheap
per-partition scalars.
"""

import numpy as np

N, NFEAT, NHID, NCLASS, NHEADS = 4096, 512, 64, 16, 8
NC = 8                      # cores
NQ = N // NC                # 512 own nodes per core
QT = NQ // 128              # 4 query tiles per core
JT = N // 128               # 32 key tiles
NCH = JT // NC              # 4 key tiles per AG chunk
ALPHA = 0.2
HW = NHID * NHEADS          # 512
HXC = NHEADS * (NHID + 1)   # 520: per-head 64 h cols + ones col
AGC2 = 18                   # AG2: 16 outh + 1 ones + 1 sdst2

# engine schedule for the 32 P-tiles of each layer-1 head sweep
_SCHED1 = ['D'] * JT
for _p in (2, 7, 13, 18, 24, 29):
    _SCHED1[_p] = 'A'
for _p in (4, 10, 16, 21, 27):
    _SCHED1[_p] = 'P'
# layer-2: 32 tiles
_SCHED2 = ['D'] * JT
for _p in (3, 11, 19, 27):
    _SCHED2[_p] = 'A'
for _p in (6, 14, 22, 29):
    _SCHED2[_p] = 'P'

_CACHE = {}


def _build_nc(no_cc=False, no_l1=False):
    import concourse.bass as bass
    import concourse.bacc as bacc
    import concourse.mybir as mybir
    import concourse.tile as tile
    from concourse.masks import make_identity

    fp32 = mybir.dt.float32
    bf16 = mybir.dt.bfloat16
    AX = mybir.AxisListType.X
    OP = mybir.AluOpType
    AF = mybir.ActivationFunctionType

    nc = bacc.Bacc()
    xT = nc.declare_dram_parameter("xT", [NFEAT, NQ], bf16, isOutput=False)
    Whr = nc.declare_dram_parameter("Whr", [NFEAT, HW], bf16, isOutput=False)
    WhrT = nc.declare_dram_parameter("WhrT", [HW, NFEAT], bf16, isOutput=False)
    Asd = nc.declare_dram_parameter("Asd", [HW, 16], bf16, isOutput=False)
    Wo = nc.declare_dram_parameter("Wo", [HW, NCLASS], bf16, isOutput=False)
    aod = nc.declare_dram_parameter("aod", [2, NCLASS], fp32, isOutput=False)
    out = nc.declare_dram_parameter("out", [NQ, NCLASS], fp32, isOutput=True)

    with tile.TileContext(nc) as tc:
        with (
            tc.tile_pool(name="const", bufs=1) as constp,
            tc.tile_pool(name="big", bufs=1) as bigp,
            tc.tile_pool(name="work", bufs=3) as workp,
            tc.tile_pool(name="pp", bufs=12) as ppool,
            tc.tile_pool(name="ps_acc", bufs=2, space="PSUM") as ps_acc,
            tc.tile_pool(name="ps_t", bufs=3, space="PSUM") as ps_t,
            tc.tile_pool(name="ps_hb", bufs=2, space="PSUM") as ps_hb,
            tc.tile_pool(name="dram", bufs=1, space="DRAM") as dramp,
        ):
            v, sc, g, te, dma = nc.vector, nc.scalar, nc.gpsimd, nc.tensor, nc.sync

            ident = constp.tile([128, 128], fp32, tag="ident")
            make_identity(nc, ident[:])
            ident_bf = constp.tile([128, 128], bf16, tag="ident_bf")
            v.tensor_copy(ident_bf[:], ident[:])
            ones1 = constp.tile([1, 128], bf16, tag="ones1")
            g.memset(ones1[:], 1.0)
            # sel[k, h*128+m] = 1 iff k == h (partition-broadcast matmuls)
            self_f = constp.tile([8, 8 * 128], fp32, tag="self_f")
            g.memset(self_f[:], 0.0)
            g.affine_select(
                out=self_f[:].rearrange("k (h m) -> k h m", m=128),
                in_=self_f[:].rearrange("k (h m) -> k h m", m=128),
                compare_op=mybir.AluOpType.not_equal,
                fill=1.0, base=0, channel_multiplier=1,
                pattern=[[-1, 8], [0, 128]])
            sel_bf = constp.tile([8, 8 * 128], bf16, tag="sel_bf")
            sc.copy(sel_bf[:], self_f[:])

            # ---- A. param loads ----
            whrT_sb = constp.tile([128, 4, NFEAT], bf16, tag="whrT_sb")
            dma.dma_start(whrT_sb[:], WhrT.rearrange("(k p) f -> p k f", p=128))
            asd_sb = constp.tile([128, 4, 16], bf16, tag="asd_sb")
            dma.dma_start(asd_sb[:], Asd.rearrange("(k p) s -> p k s", p=128))
            xT_sb = constp.tile([128, 4, NQ], bf16, tag="xT_sb")
            dma.dma_start(xT_sb[:], xT.rearrange("(k p) q -> p k q", p=128))
            whr_sb = constp.tile([128, 4, HW], bf16, tag="whr_sb")
            dma.dma_start(whr_sb[:], Whr.rearrange("(k p) c -> p k c", p=128))
            wo_sb = constp.tile([128, 4, 16], bf16, tag="wo_sb")
            dma.dma_start(wo_sb[:], Wo.rearrange("(k p) s -> p k s", p=128))
            aos_b = constp.tile([128, 16], fp32, tag="aos_b")
            dma.dma_start(aos_b[:], aod[0:1, :].to_broadcast((128, 16)))
            aod_b = constp.tile([128, 16], fp32, tag="aod_b")
            dma.dma_start(aod_b[:], aod[1:2, :].to_broadcast((128, 16)))

            ag1h_in = dramp.tile([NQ, HXC], bf16, tag="ag1h_in")
            ag1h_out = dramp.tile([N, HXC], bf16, tag="ag1h_out",
                                  addr_space="Local" if no_cc else "Shared")
            ag1s_in = dramp.tile([NQ, 8], fp32, tag="ag1s_in")
            ag1s_out = dramp.tile([N, 8], fp32, tag="ag1s_out",
                                  addr_space="Local" if no_cc else "Shared")
            ag2_in = dramp.tile([NQ, AGC2], fp32, tag="ag2_in")
            ag2_out = dramp.tile([N, AGC2], fp32, tag="ag2_out",
                                 addr_space="Local" if no_cc else "Shared")

            # ---- B. Wa_feat = Whr @ Asd  (score weight cols in x-basis) ----
            wa_ps = ps_t.tile([16, NFEAT], fp32, tag="tp", name="wa_ps")
            for k in range(4):
                te.matmul(wa_ps[:], asd_sb[:, k, :], whrT_sb[:, k, :],
                          start=(k == 0), stop=(k == 3))
            waT_sb = constp.tile([16, NFEAT], bf16, tag="waT_sb")
            sc.copy(waT_sb[:], wa_ps[:])
            waf_ps = ps_t.tile([128, 4, 16], bf16, tag="tp", name="waf_ps")
            for k in range(4):
                te.transpose(waf_ps[:, k, :], waT_sb[:, k * 128:(k + 1) * 128],
                             ident_bf[0:16, 0:16])
            wa_f = constp.tile([128, 4, 16], bf16, tag="wa_f")
            v.tensor_copy(wa_f[:], waf_ps[:])

            # ---- C. h_own key-major + s_own; stage AG1 ----
            stg = [bigp.tile([128, HXC], bf16, tag=f"stg{qt}", name=f"stg{qt}")
                   for qt in range(QT)]
            stgs = [bigp.tile([128, 16], fp32, tag=f"stgs{qt}",
                              name=f"stgs{qt}") for qt in range(QT)]
            for qt in range(QT):
                h_ps = ps_acc.tile([128, HW], fp32, tag="acc", name="h_ps")
                s_ps = ps_t.tile([128, 16], fp32, tag="tp", name="s_ps")
                for k in range(4):
                    lhs = xT_sb[:, k, qt * 128:(qt + 1) * 128]
                    te.matmul(h_ps[:], lhs, whr_sb[:, k, :],
                              start=(k == 0), stop=(k == 3))
                    te.matmul(s_ps[:], lhs, wa_f[:, k, :],
                              start=(k == 0), stop=(k == 3))
                eng = sc if qt % 2 else v
                eng_c = eng.copy if qt % 2 else eng.tensor_copy
                eng_c(stg[qt][:].rearrange("p (h c) -> p h c", c=65)[:, :, 0:64],
                      h_ps[:].rearrange("p (h c) -> p h c", c=64))
                g.memset(
                    stg[qt][:].rearrange("p (h c) -> p h c", c=65)[:, :, 64:65],
                    1.0)
                v.tensor_copy(stgs[qt][:], s_ps[:])
                dma.dma_start(ag1h_in[qt * 128:(qt + 1) * 128, :], stg[qt][:])
                dma.dma_start(ag1s_in[qt * 128:(qt + 1) * 128, :],
                              stgs[qt][:, 8:16])

            # ---- D. w panel (own s_src): transpose, exp, broadcast ----
            s_fm = ps_t.tile([16, NQ], fp32, tag="tp", name="s_fm")
            for qt in range(QT):
                te.transpose(s_fm[:, qt * 128:(qt + 1) * 128], stgs[qt][:],
                             ident[0:128, 0:128])
            w_bf = constp.tile([8, NQ], bf16, tag="w_bf")
            sc.activation(w_bf[:], s_fm[0:8, :], AF.Exp, scale=-0.8)
            wb_all = constp.tile([128, NHEADS, NQ], bf16, tag="wb_all")
            for h in range(NHEADS):
                wb_ps = ps_t.tile([128, NQ], fp32, tag="tp", name="wb_ps")
                te.matmul(wb_ps[:], sel_bf[:, h * 128:(h + 1) * 128], w_bf[:],
                          start=True, stop=True)
                eng = (v.tensor_copy, sc.copy, sc.copy)[h % 3]
                eng(wb_all[:, h, :], wb_ps[:])

            # ---- E. AllGather 1 ----
            if no_cc:
                for r in range(NC):
                    dma.dma_start(ag1s_out[r * NQ:(r + 1) * NQ, :], ag1s_in[:])
            else:
                g.collective_compute(
                    "AllGather", OP.bypass,
                    ins=[ag1s_in.opt()], outs=[ag1s_out.opt()],
                    replica_groups=[list(range(NC))],
                )

            # ---- F. key-side score panels ----
            sd_pan = constp.tile([128, JT * 8], fp32, tag="sd_pan")
            dma.dma_start(
                sd_pan[:].rearrange("p (t h) -> p t h", h=8),
                ag1s_out.rearrange("(t p) h -> p t h", p=128))
            b_all = constp.tile([128, JT * 8], fp32, tag="b_all")
            sc.activation(b_all[:], sd_pan[:], AF.Exp)
            d_all = constp.tile([128, JT * 8], fp32, tag="d_all")
            sc.activation(d_all[:], sd_pan[:], AF.Exp, scale=ALPHA)
            nb_all = constp.tile([128, JT * 8], fp32, tag="nb_all")
            v.tensor_scalar(nb_all[:], b_all[:], -1.0, None, OP.mult)
            b_bf = constp.tile([128, JT * 8], bf16, tag="b_bf")
            v.tensor_copy(b_bf[:], b_all[:])

            if no_cc:
                for r in range(NC):
                    dma.dma_start(ag1h_out[r * NQ:(r + 1) * NQ, :], ag1h_in[:])
            else:
                g.collective_compute(
                    "AllGather", OP.bypass,
                    ins=[ag1h_in.opt()], outs=[ag1h_out.opt()],
                    replica_groups=[list(range(NC))],
                )

            # ---- G. hx chunk loads (gated per AG chunk) ----
            hx = []
            for c in range(NC):
                t = bigp.tile([128, NCH, HXC], bf16, tag=f"hx{c}",
                              name=f"hx{c}")
                dma.dma_start(
                    t[:],
                    ag1h_out[c * NQ:(c + 1) * NQ, :].rearrange(
                        "(a p) x -> p a x", p=128))
                hx.append(t)

            def hx_slice(jt, h, w):
                return hx[jt // NCH][:, jt % NCH, h * 65:h * 65 + w]

            # ---- H. layer-1 attention ----
            xc_pre = [bigp.tile([128, HW], fp32, tag=f"xc{qc}",
                                name=f"xc{qc}") for qc in range(QT)]
            r_pan = constp.tile([128, NHEADS, QT], fp32, tag="r_pan")

            for h in range(NHEADS if not no_l1 else 0):
                acc = ps_acc.tile([128, QT, 65], fp32, tag="acc", name="acc")
                act_jts = []
                hb_ps = ps_hb.tile([1, 65], fp32, tag="hb", name="hb_ps")
                for idx in range(JT):
                    jt = (h * NCH + idx) % JT
                    col = slice(jt * 8 + h, jt * 8 + h + 1)
                    e = _SCHED1[idx]
                    pt = ppool.tile([128, NQ], bf16, tag="pt", name="pt")
                    if e == 'A':
                        sc.activation(pt[:], wb_all[:, h, :], AF.Relu,
                                      bias=nb_all[:, col], scale=d_all[:, col])
                        te.matmul(hb_ps[:], b_bf[:, col], hx_slice(jt, h, 65),
                                  start=(not act_jts), stop=(idx == 29))
                        act_jts.append(jt)
                    else:
                        eng = v if e == 'D' else g
                        eng.tensor_scalar(pt[:], wb_all[:, h, :],
                                          d_all[:, col], b_all[:, col],
                                          OP.mult, OP.max)
                    for qc in range(QT):
                        te.matmul(acc[:, qc, :],
                                  pt[:, qc * 128:(qc + 1) * 128],
                                  hx_slice(jt, h, 65),
                                  start=(idx == 0), stop=False)
                # rank-1 correction for the ACT-produced tiles: acc += 1 (x) hb
                hb_sb = workp.tile([1, 65], bf16, tag="hb_sb", bufs=2)
                v.tensor_copy(hb_sb[:], hb_ps[:])
                for qc in range(QT):
                    te.matmul(acc[:, qc, :], ones1[:], hb_sb[:],
                              start=False, stop=True)
                # normalize: r = 1/den, xc_pre[:, h*64:] = f * r
                v.reciprocal(r_pan[:, h, :], acc[:, :, 64])
                for qc in range(QT):
                    k = (h * QT + qc) % 3
                    dst = xc_pre[qc][:, h * 64:(h + 1) * 64]
                    if k == 0:
                        sc.activation(dst, acc[:, qc, 0:64], AF.Copy,
                                      scale=r_pan[:, h, qc:qc + 1])
                    elif k == 1:
                        v.tensor_scalar(dst, acc[:, qc, 0:64],
                                        r_pan[:, h, qc:qc + 1], None, OP.mult)
                    else:
                        g.tensor_scalar(dst, acc[:, qc, 0:64],
                                        r_pan[:, h, qc:qc + 1], None, OP.mult)

            if no_l1:
                for qc in range(QT):
                    g.memset(xc_pre[qc][:], 0.5)

            # ---- I. elu (fp32), transpose to feature-major bf16 ----
            xcT = [bigp.tile([128, QT, 128], bf16, tag=f"xcT{fc}",
                             name=f"xcT{fc}") for fc in range(4)]
            for qc in range(QT):
                e1 = workp.tile([128, HW], fp32, tag="elu_e", name="e1")
                sc.activation(e1[:], xc_pre[qc][:], AF.Exp)
                v.tensor_scalar(e1[:], e1[:], 1.0, 0.0, OP.subtract, OP.min)
                (g if qc % 2 else v).tensor_tensor(
                    xc_pre[qc][:], xc_pre[qc][:], e1[:], OP.max)
            for fc in range(4):
                tp = ps_t.tile([128, QT, 128], fp32, tag="tp", name="tp_xc")
                for qc in range(QT):
                    te.transpose(tp[:, qc, :],
                                 xc_pre[qc][:, fc * 128:(fc + 1) * 128],
                                 ident[:])
                eng = (v.tensor_copy, sc.copy, sc.copy)[fc % 3]
                eng(xcT[fc][:], tp[:])

            # ---- J. project to outh, stage AG2 ----
            w2tmp = constp.tile([128, QT], fp32, tag="w2tmp")
            for qt in range(QT):
                o_ps = ps_t.tile([128, 16], fp32, tag="tp", name="o_ps")
                for fc in range(4):
                    te.matmul(o_ps[:], xcT[fc][:, qt, :], wo_sb[:, fc, :],
                              start=(fc == 0), stop=(fc == 3))
                stg2 = bigp.tile([128, AGC2], fp32, tag=f"stg2_{qt}",
                                 name=f"stg2_{qt}")
                v.tensor_copy(stg2[:, 0:16], o_ps[:])
                g.memset(stg2[:, 16:17], 1.0)
                tmp = workp.tile([128, 16], fp32, tag="sdtmp")
                v.scalar_tensor_tensor(tmp[:], o_ps[:], 1.0, aod_b[:],
                                       OP.mult, OP.mult,
                                       accum_out=stg2[:, 17:18])
                tmp2 = workp.tile([128, 16], fp32, tag="sdtmp2")
                v.scalar_tensor_tensor(tmp2[:], o_ps[:], 1.0, aos_b[:],
                                       OP.mult, OP.mult,
                                       accum_out=w2tmp[:, qt:qt + 1])
                dma.dma_start(ag2_in[qt * 128:(qt + 1) * 128, :], stg2[:])

            # ---- K. w2 broadcast panel ----
            w2e = constp.tile([128, QT], fp32, tag="w2e")
            sc.activation(w2e[:], w2tmp[:], AF.Exp, scale=-0.8)
            w2tp = ps_hb.tile([QT, 128], fp32, tag="hb", name="w2tp")
            te.transpose(w2tp[:], w2e[:], ident[:])
            w2T_bf = constp.tile([QT, 128], bf16, tag="w2T_bf")
            v.tensor_copy(w2T_bf[:], w2tp[:])
            w2b_ps = ps_t.tile([128, QT, 128], fp32, tag="tp", name="w2b_ps")
            for qt in range(QT):
                te.matmul(w2b_ps[:, qt, :],
                          sel_bf[0:QT, qt * 128:(qt + 1) * 128], w2T_bf[:],
                          start=True, stop=True)
            w2b = constp.tile([128, NQ], bf16, tag="w2b")
            sc.copy(w2b[:], w2b_ps[:].rearrange("p a q -> p (a q)"))

            # ---- L. AllGather 2 + panels ----
            if no_cc:
                for r in range(NC):
                    dma.dma_start(ag2_out[r * NQ:(r + 1) * NQ, :], ag2_in[:])
            else:
                g.collective_compute(
                    "AllGather", OP.bypass,
                    ins=[ag2_in.opt()], outs=[ag2_out.opt()],
                    replica_groups=[list(range(NC))],
                )
            pan2 = constp.tile([128, JT, AGC2], fp32, tag="pan2")
            dma.dma_start(pan2[:],
                          ag2_out.rearrange("(t p) c -> p t c", p=128))
            hx2 = constp.tile([128, JT, 17], bf16, tag="hx2")
            sc.copy(hx2[:], pan2[:, :, 0:17])
            b2 = constp.tile([128, JT], fp32, tag="b2")
            sc.activation(b2[:], pan2[:, :, 17], AF.Exp)
            d2 = constp.tile([128, JT], fp32, tag="d2")
            sc.activation(d2[:], pan2[:, :, 17], AF.Exp, scale=ALPHA)
            nb2 = constp.tile([128, JT], fp32, tag="nb2")
            v.tensor_scalar(nb2[:], b2[:], -1.0, None, OP.mult)
            b2_bf = constp.tile([128, JT], bf16, tag="b2_bf")
            v.tensor_copy(b2_bf[:], b2[:])

            # ---- M. layer-2 attention ----
            acc2 = ps_acc.tile([128, QT, 17], fp32, tag="acc", name="acc2")
            hb2_ps = ps_hb.tile([1, 17], fp32, tag="hb", name="hb2_ps")
            n_act2 = 0
            for jt in range(JT):
                e = _SCHED2[jt]
                pt = ppool.tile([128, NQ], bf16, tag="pt", name="pt2")
                if e == 'A':
                    sc.activation(pt[:], w2b[:], AF.Relu,
                                  bias=nb2[:, jt:jt + 1], scale=d2[:, jt:jt + 1])
                    te.matmul(hb2_ps[:], b2_bf[:, jt:jt + 1], hx2[:, jt, :],
                              start=(n_act2 == 0), stop=(jt == 27))
                    n_act2 += 1
                else:
                    eng = v if e == 'D' else g
                    eng.tensor_scalar(pt[:], w2b[:], d2[:, jt:jt + 1],
                                      b2[:, jt:jt + 1], OP.mult, OP.max)
                for qc in range(QT):
                    te.matmul(acc2[:, qc, :], pt[:, qc * 128:(qc + 1) * 128],
                              hx2[:, jt, :], start=(jt == 0), stop=False)
            hb2_sb = workp.tile([1, 17], bf16, tag="hb2_sb")
            v.tensor_copy(hb2_sb[:], hb2_ps[:])
            for qc in range(QT):
                te.matmul(acc2[:, qc, :], ones1[:], hb2_sb[:],
                          start=False, stop=True)

            # ---- N. normalize, elu, log_softmax, store ----
            r2 = workp.tile([128, QT], fp32, tag="r2")
            v.reciprocal(r2[:], acc2[:, :, 16])
            o_all = workp.tile([128, QT, 16], fp32, tag="o_all")
            for qc in range(QT):
                sc.activation(o_all[:, qc, :], acc2[:, qc, 0:16], AF.Copy,
                              scale=r2[:, qc:qc + 1])
            e2 = workp.tile([128, QT, 16], fp32, tag="e2")
            sc.activation(e2[:].rearrange("p a c -> p (a c)"),
                          o_all[:].rearrange("p a c -> p (a c)"), AF.Exp)
            v.tensor_scalar(e2[:].rearrange("p a c -> p (a c)"),
                            e2[:].rearrange("p a c -> p (a c)"),
                            1.0, 0.0, OP.subtract, OP.min)
            v.tensor_tensor(o_all[:].rearrange("p a c -> p (a c)"),
                            o_all[:].rearrange("p a c -> p (a c)"),
                            e2[:].rearrange("p a c -> p (a c)"), OP.max)
            ee = workp.tile([128, QT, 16], fp32, tag="ee")
            sc.activation(ee[:].rearrange("p a c -> p (a c)"),
                          o_all[:].rearrange("p a c -> p (a c)"), AF.Exp)
            s2s = workp.tile([128, QT], fp32, tag="s2s")
            v.tensor_reduce(s2s[:], ee[:], AX, OP.add)
            lse = workp.tile([128, QT], fp32, tag="lse")
            sc.activation(lse[:], s2s[:], AF.Ln)
            fin = workp.tile([128, QT, 16], fp32, tag="fin")
            for qc in range(QT):
                v.tensor_scalar(fin[:, qc, :], o_all[:, qc, :],
                                lse[:, qc:qc + 1], None, OP.subtract)
            dma.dma_start(out.rearrange("(a p) c -> p a c", p=128), fin[:])

    nc.finalize()
    return nc


def _get_compiled(no_cc=False, no_l1=False):
    key = ("nc", no_cc, no_l1)
    if key not in _CACHE:
        _CACHE[key] = _build_nc(no_cc=no_cc, no_l1=no_l1)
    return _CACHE[key]


def kernel(x, Wh, ah, Wo, ao):
    import ml_dtypes
    from concourse.bass_utils import run_bass_kernel_spmd

    bf = ml_dtypes.bfloat16
    nc = _get_compiled()
    x = np.asarray(x, np.float32)
    Wh = np.asarray(Wh, np.float32)
    ah = np.asarray(ah, np.float32)
    Wo = np.asarray(Wo, np.float32)
    ao = np.asarray(ao, np.float32)

    # host-side relayouts (no math): head-major weight matrix, its transpose,
    # block-diag score matrix, split ao
    Whr = np.ascontiguousarray(
        Wh.transpose(1, 0, 2).reshape(NFEAT, HW))          # [512, 512]
    WhrT = np.ascontiguousarray(Whr.T)
    Asd = np.zeros((HW, 16), np.float32)
    for h in range(NHEADS):
        Asd[h * NHID:(h + 1) * NHID, h] = ah[h, :NHID]      # src
        Asd[h * NHID:(h + 1) * NHID, 8 + h] = ah[h, NHID:]  # dst
    aod = np.stack([ao[:NCLASS], ao[NCLASS:]])              # [2, 16]

    Whr_b = Whr.astype(bf)
    WhrT_b = WhrT.astype(bf)
    Asd_b = Asd.astype(bf)
    Wo_b = np.ascontiguousarray(Wo).astype(bf)

    in_maps = []
    for i in range(NC):
        in_maps.append({
            "xT": np.ascontiguousarray(x[i * NQ:(i + 1) * NQ].T).astype(bf),
            "Whr": Whr_b, "WhrT": WhrT_b, "Asd": Asd_b,
            "Wo": Wo_b, "aod": aod,
        })
    res = run_bass_kernel_spmd(nc, in_maps, list(range(NC)))
    return np.concatenate([res.results[i]["out"] for i in range(NC)], 0)


# revision 7
# speedup vs baseline: 1.3162x; 1.0064x over previous
"""GAT (2-layer, 8-head) fused Bass kernel for 8 trn2 NeuronCores.

Sharding: nodes (rows of x) split 512/core. Per core: h computed key-major
with fused score columns; h (bf16) + s_dst (fp32) AllGather'd; each core
computes its 512xN attention block for all 8 heads; layer-1 output projected
and AllGather'd (18 fp32 cols); each core computes its 512xN layer-2 block
and the final log_softmax rows.

Key algebra: with s_i = h_i . a_src, d_j = h_j . a_dst,
  exp(leakyrelu(s_i + d_j)) = max(exp(s_i)exp(d_j), exp(.2 s_i)exp(.2 d_j))
and softmax over j is invariant to any per-i scale, so the attention
numerator is P[j,i] = max(b_j, w_i * dd_j) with b_j = exp(d_j),
w_i = exp(-0.8 s_i), dd_j = exp(0.2 d_j).

P tiles [128 keys, 512 queries] are produced on three engines:
  DVE/Pool: tensor_scalar (mult, max) -> P
  ACT:      relu(dd_j * w_i - b_j) = P - b_j, single activation op; the
            missing rank-1 term hb[c] = sum_j b_j hx[j,c] over ACT-tiles is
            added back into the PSUM accumulation via two tiny matmuls.
Attention matmuls run with the P chunk [128k x 128q] as the *stationary*
operand and the per-head hx block [128, 65] (64 h cols + ones) as the
*moving* operand: 65 columns/matmul instead of 512 -> ~2x less PE time,
and the output lands query-major so normalize/elu/log_softmax use cheap
per-partition scalars.
"""

import numpy as np

N, NFEAT, NHID, NCLASS, NHEADS = 4096, 512, 64, 16, 8
NC = 8                      # cores
NQ = N // NC                # 512 own nodes per core
QT = NQ // 128              # 4 query tiles per core
JT = N // 128               # 32 key tiles
NCH = JT // NC              # 4 key tiles per AG chunk
ALPHA = 0.2
HW = NHID * NHEADS          # 512
HXC = NHEADS * (NHID + 1)   # 520: per-head 64 h cols + ones col
AGC2 = 18                   # AG2: 16 outh + 1 ones + 1 sdst2

# engine schedule for the 32 P-tiles of each layer-1 head sweep
_SCHED1 = ['D'] * JT
for _p in (2, 7, 13, 18, 24, 29):
    _SCHED1[_p] = 'A'
for _p in (4, 10, 16, 21, 27):
    _SCHED1[_p] = 'P'
# layer-2: 32 tiles
_SCHED2 = ['D'] * JT
for _p in (3, 11, 19, 27):
    _SCHED2[_p] = 'A'
for _p in (6, 14, 22, 29):
    _SCHED2[_p] = 'P'

_CACHE = {}


def _build_nc(no_cc=False, no_l1=False):
    import concourse.bass as bass
    import concourse.bacc as bacc
    import concourse.mybir as mybir
    import concourse.tile as tile
    from concourse.masks import make_identity

    fp32 = mybir.dt.float32
    bf16 = mybir.dt.bfloat16
    AX = mybir.AxisListType.X
    OP = mybir.AluOpType
    AF = mybir.ActivationFunctionType

    nc = bacc.Bacc()
    xT = nc.declare_dram_parameter("xT", [NFEAT, NQ], bf16, isOutput=False)
    Whr = nc.declare_dram_parameter("Whr", [NFEAT, HW], bf16, isOutput=False)
    WhrT = nc.declare_dram_parameter("WhrT", [HW, NFEAT], bf16, isOutput=False)
    Asd = nc.declare_dram_parameter("Asd", [HW, 16], bf16, isOutput=False)
    Wo = nc.declare_dram_parameter("Wo", [HW, NCLASS], bf16, isOutput=False)
    aod = nc.declare_dram_parameter("aod", [2, NCLASS], fp32, isOutput=False)
    out = nc.declare_dram_parameter("out", [NQ, NCLASS], fp32, isOutput=True)

    with tile.TileContext(nc) as tc:
        with (
            tc.tile_pool(name="const", bufs=1) as constp,
            tc.tile_pool(name="big", bufs=1) as bigp,
            tc.tile_pool(name="work", bufs=3) as workp,
            tc.tile_pool(name="pp", bufs=12) as ppool,
            tc.tile_pool(name="ps_acc", bufs=2, space="PSUM") as ps_acc,
            tc.tile_pool(name="ps_t", bufs=3, space="PSUM") as ps_t,
            tc.tile_pool(name="ps_hb", bufs=2, space="PSUM") as ps_hb,
            tc.tile_pool(name="dram", bufs=1, space="DRAM") as dramp,
        ):
            v, sc, g, te, dma = nc.vector, nc.scalar, nc.gpsimd, nc.tensor, nc.sync

            ident = constp.tile([128, 128], fp32, tag="ident")
            make_identity(nc, ident[:])
            ident_bf = constp.tile([128, 128], bf16, tag="ident_bf")
            v.tensor_copy(ident_bf[:], ident[:])
            ones1 = constp.tile([1, 128], bf16, tag="ones1")
            g.memset(ones1[:], 1.0)
            # sel[k, h*128+m] = 1 iff k == h (partition-broadcast matmuls)
            self_f = constp.tile([8, 8 * 128], fp32, tag="self_f")
            g.memset(self_f[:], 0.0)
            g.affine_select(
                out=self_f[:].rearrange("k (h m) -> k h m", m=128),
                in_=self_f[:].rearrange("k (h m) -> k h m", m=128),
                compare_op=mybir.AluOpType.not_equal,
                fill=1.0, base=0, channel_multiplier=1,
                pattern=[[-1, 8], [0, 128]])
            sel_bf = constp.tile([8, 8 * 128], bf16, tag="sel_bf")
            sc.copy(sel_bf[:], self_f[:])

            # ---- A. param loads ----
            whrT_sb = constp.tile([128, 4, NFEAT], bf16, tag="whrT_sb")
            dma.dma_start(whrT_sb[:], WhrT.rearrange("(k p) f -> p k f", p=128))
            asd_sb = constp.tile([128, 4, 16], bf16, tag="asd_sb")
            dma.dma_start(asd_sb[:], Asd.rearrange("(k p) s -> p k s", p=128))
            xT_sb = constp.tile([128, 4, NQ], bf16, tag="xT_sb")
            dma.dma_start(xT_sb[:], xT.rearrange("(k p) q -> p k q", p=128))
            whr_sb = constp.tile([128, 4, HW], bf16, tag="whr_sb")
            dma.dma_start(whr_sb[:], Whr.rearrange("(k p) c -> p k c", p=128))
            wo_sb = constp.tile([128, 4, 16], bf16, tag="wo_sb")
            dma.dma_start(wo_sb[:], Wo.rearrange("(k p) s -> p k s", p=128))
            aos_b = constp.tile([128, 16], fp32, tag="aos_b")
            dma.dma_start(aos_b[:], aod[0:1, :].to_broadcast((128, 16)))
            aod_b = constp.tile([128, 16], fp32, tag="aod_b")
            dma.dma_start(aod_b[:], aod[1:2, :].to_broadcast((128, 16)))

            ag1h_in = dramp.tile([NQ, HXC], bf16, tag="ag1h_in")
            ag1h_out = dramp.tile([N, HXC], bf16, tag="ag1h_out",
                                  addr_space="Local" if no_cc else "Shared")
            ag1s_in = dramp.tile([NQ, 8], fp32, tag="ag1s_in")
            ag1s_out = dramp.tile([N, 8], fp32, tag="ag1s_out",
                                  addr_space="Local" if no_cc else "Shared")
            ag2_in = dramp.tile([NQ, AGC2], fp32, tag="ag2_in")
            ag2_out = dramp.tile([N, AGC2], fp32, tag="ag2_out",
                                 addr_space="Local" if no_cc else "Shared")

            # ---- B. Wa_feat = Whr @ Asd  (score weight cols in x-basis) ----
            wa_ps = ps_t.tile([16, NFEAT], fp32, tag="tp", name="wa_ps")
            for k in range(4):
                te.matmul(wa_ps[:], asd_sb[:, k, :], whrT_sb[:, k, :],
                          start=(k == 0), stop=(k == 3))
            waT_sb = constp.tile([16, NFEAT], bf16, tag="waT_sb")
            sc.copy(waT_sb[:], wa_ps[:])
            waf_ps = ps_t.tile([128, 4, 16], bf16, tag="tp", name="waf_ps")
            for k in range(4):
                te.transpose(waf_ps[:, k, :], waT_sb[:, k * 128:(k + 1) * 128],
                             ident_bf[0:16, 0:16])
            wa_f = constp.tile([128, 4, 16], bf16, tag="wa_f")
            v.tensor_copy(wa_f[:], waf_ps[:])

            # ---- C. h_own key-major + s_own; stage AG1 ----
            stg = [bigp.tile([128, HXC], bf16, tag=f"stg{qt}", name=f"stg{qt}")
                   for qt in range(QT)]
            stgs = [bigp.tile([128, 16], fp32, tag=f"stgs{qt}",
                              name=f"stgs{qt}") for qt in range(QT)]
            for qt in range(QT):
                h_ps = ps_acc.tile([128, HW], fp32, tag="acc", name="h_ps")
                s_ps = ps_t.tile([128, 16], fp32, tag="tp", name="s_ps")
                for k in range(4):
                    lhs = xT_sb[:, k, qt * 128:(qt + 1) * 128]
                    te.matmul(h_ps[:], lhs, whr_sb[:, k, :],
                              start=(k == 0), stop=(k == 3))
                    te.matmul(s_ps[:], lhs, wa_f[:, k, :],
                              start=(k == 0), stop=(k == 3))
                eng = sc if qt % 2 else v
                eng_c = eng.copy if qt % 2 else eng.tensor_copy
                eng_c(stg[qt][:].rearrange("p (h c) -> p h c", c=65)[:, :, 0:64],
                      h_ps[:].rearrange("p (h c) -> p h c", c=64))
                g.memset(
                    stg[qt][:].rearrange("p (h c) -> p h c", c=65)[:, :, 64:65],
                    1.0)
                v.tensor_copy(stgs[qt][:], s_ps[:])
                dma.dma_start(ag1h_in[qt * 128:(qt + 1) * 128, :], stg[qt][:])
                dma.dma_start(ag1s_in[qt * 128:(qt + 1) * 128, :],
                              stgs[qt][:, 8:16])

            # ---- D. w panel (own s_src): transpose, exp, broadcast ----
            s_fm = ps_t.tile([16, NQ], fp32, tag="tp", name="s_fm")
            for qt in range(QT):
                te.transpose(s_fm[:, qt * 128:(qt + 1) * 128], stgs[qt][:],
                             ident[0:128, 0:128])
            w_bf = constp.tile([8, NQ], bf16, tag="w_bf")
            sc.activation(w_bf[:], s_fm[0:8, :], AF.Exp, scale=-0.8)
            wb_all = constp.tile([128, NHEADS, NQ], bf16, tag="wb_all")
            for h in range(NHEADS):
                wb_ps = ps_t.tile([128, NQ], fp32, tag="tp", name="wb_ps")
                te.matmul(wb_ps[:], sel_bf[:, h * 128:(h + 1) * 128], w_bf[:],
                          start=True, stop=True)
                eng = (v.tensor_copy, sc.copy, sc.copy)[h % 3]
                eng(wb_all[:, h, :], wb_ps[:])

            # ---- E. AllGather 1 ----
            if no_cc:
                for r in range(NC):
                    dma.dma_start(ag1s_out[r * NQ:(r + 1) * NQ, :], ag1s_in[:])
            else:
                g.collective_compute(
                    "AllGather", OP.bypass,
                    ins=[ag1s_in.opt()], outs=[ag1s_out.opt()],
                    replica_groups=[list(range(NC))],
                )

            # ---- F. key-side score panels ----
            sd_pan = constp.tile([128, JT * 8], fp32, tag="sd_pan")
            dma.dma_start(
                sd_pan[:].rearrange("p (t h) -> p t h", h=8),
                ag1s_out.rearrange("(t p) h -> p t h", p=128))
            b_all = constp.tile([128, JT * 8], fp32, tag="b_all")
            sc.activation(b_all[:], sd_pan[:], AF.Exp)
            d_all = constp.tile([128, JT * 8], fp32, tag="d_all")
            sc.activation(d_all[:], sd_pan[:], AF.Exp, scale=ALPHA)
            nb_all = constp.tile([128, JT * 8], fp32, tag="nb_all")
            v.tensor_scalar(nb_all[:], b_all[:], -1.0, None, OP.mult)
            b_bf = constp.tile([128, JT * 8], bf16, tag="b_bf")
            v.tensor_copy(b_bf[:], b_all[:])

            if no_cc:
                for r in range(NC):
                    dma.dma_start(ag1h_out[r * NQ:(r + 1) * NQ, :], ag1h_in[:])
            else:
                g.collective_compute(
                    "AllGather", OP.bypass,
                    ins=[ag1h_in.opt()], outs=[ag1h_out.opt()],
                    replica_groups=[list(range(NC))],
                )

            # ---- G. hx chunk loads (gated per AG chunk) ----
            hx = []
            for c in range(NC):
                t = bigp.tile([128, NCH, HXC], bf16, tag=f"hx{c}",
                              name=f"hx{c}")
                dma.dma_start(
                    t[:],
                    ag1h_out[c * NQ:(c + 1) * NQ, :].rearrange(
                        "(a p) x -> p a x", p=128))
                hx.append(t)

            def hx_slice(jt, h, w):
                return hx[jt // NCH][:, jt % NCH, h * 65:h * 65 + w]

            # ---- H. layer-1 attention ----
            xc_pre = [bigp.tile([128, HW], fp32, tag=f"xc{qc}",
                                name=f"xc{qc}") for qc in range(QT)]
            r_pan = constp.tile([128, NHEADS, QT], fp32, tag="r_pan")

            for h in range(NHEADS if not no_l1 else 0):
                acc = ps_acc.tile([128, QT, 65], fp32, tag="acc", name="acc")
                act_jts = []
                hb_ps = ps_hb.tile([1, 65], fp32, tag="hb", name="hb_ps")
                for idx in range(JT):
                    jt = (h * NCH + idx) % JT
                    col = slice(jt * 8 + h, jt * 8 + h + 1)
                    e = _SCHED1[idx]
                    pt = ppool.tile([128, NQ], bf16, tag="pt", name="pt")
                    if e == 'A':
                        sc.activation(pt[:], wb_all[:, h, :], AF.Relu,
                                      bias=nb_all[:, col], scale=d_all[:, col])
                        te.matmul(hb_ps[:], b_bf[:, col], hx_slice(jt, h, 65),
                                  start=(not act_jts), stop=(idx == 29))
                        act_jts.append(jt)
                    else:
                        eng = v if e == 'D' else g
                        eng.tensor_scalar(pt[:], wb_all[:, h, :],
                                          d_all[:, col], b_all[:, col],
                                          OP.mult, OP.max)
                    for qc in range(QT):
                        te.matmul(acc[:, qc, :],
                                  pt[:, qc * 128:(qc + 1) * 128],
                                  hx_slice(jt, h, 65),
                                  start=(idx == 0), stop=False)
                # rank-1 correction for the ACT-produced tiles: acc += 1 (x) hb
                hb_sb = workp.tile([1, 65], bf16, tag="hb_sb", bufs=2)
                v.tensor_copy(hb_sb[:], hb_ps[:])
                for qc in range(QT):
                    te.matmul(acc[:, qc, :], ones1[:], hb_sb[:],
                              start=False, stop=True)
                # normalize: r = 1/den, xc_pre[:, h*64:] = f * r
                v.reciprocal(r_pan[:, h, :], acc[:, :, 64])
                for qc in range(QT):
                    dst = xc_pre[qc][:, h * 64:(h + 1) * 64]
                    if (h * QT + qc) % 2:
                        sc.activation(dst, acc[:, qc, 0:64], AF.Copy,
                                      scale=r_pan[:, h, qc:qc + 1])
                    else:
                        v.tensor_scalar(dst, acc[:, qc, 0:64],
                                        r_pan[:, h, qc:qc + 1], None, OP.mult)

            if no_l1:
                for qc in range(QT):
                    g.memset(xc_pre[qc][:], 0.5)

            # ---- I. elu (fp32), transpose to feature-major bf16 ----
            xcT = [bigp.tile([128, QT, 128], bf16, tag=f"xcT{fc}",
                             name=f"xcT{fc}") for fc in range(4)]
            for qc in range(QT):
                e1 = workp.tile([128, HW], fp32, tag="elu_e", name="e1")
                sc.activation(e1[:], xc_pre[qc][:], AF.Exp)
                v.tensor_scalar(e1[:], e1[:], 1.0, 0.0, OP.subtract, OP.min)
                v.tensor_tensor(xc_pre[qc][:], xc_pre[qc][:], e1[:], OP.max)
            for fc in range(4):
                tp = ps_t.tile([128, QT, 128], fp32, tag="tp", name="tp_xc")
                for qc in range(QT):
                    te.transpose(tp[:, qc, :],
                                 xc_pre[qc][:, fc * 128:(fc + 1) * 128],
                                 ident[:])
                eng = (v.tensor_copy, sc.copy, sc.copy)[fc % 3]
                eng(xcT[fc][:], tp[:])

            # ---- J. project to outh, stage AG2 ----
            w2tmp = constp.tile([128, QT], fp32, tag="w2tmp")
            for qt in range(QT):
                o_ps = ps_t.tile([128, 16], fp32, tag="tp", name="o_ps")
                for fc in range(4):
                    te.matmul(o_ps[:], xcT[fc][:, qt, :], wo_sb[:, fc, :],
                              start=(fc == 0), stop=(fc == 3))
                stg2 = bigp.tile([128, AGC2], fp32, tag=f"stg2_{qt}",
                                 name=f"stg2_{qt}")
                v.tensor_copy(stg2[:, 0:16], o_ps[:])
                g.memset(stg2[:, 16:17], 1.0)
                tmp = workp.tile([128, 16], fp32, tag="sdtmp")
                v.scalar_tensor_tensor(tmp[:], o_ps[:], 1.0, aod_b[:],
                                       OP.mult, OP.mult,
                                       accum_out=stg2[:, 17:18])
                tmp2 = workp.tile([128, 16], fp32, tag="sdtmp2")
                v.scalar_tensor_tensor(tmp2[:], o_ps[:], 1.0, aos_b[:],
                                       OP.mult, OP.mult,
                                       accum_out=w2tmp[:, qt:qt + 1])
                dma.dma_start(ag2_in[qt * 128:(qt + 1) * 128, :], stg2[:])

            # ---- K. w2 broadcast panel ----
            w2e = constp.tile([128, QT], fp32, tag="w2e")
            sc.activation(w2e[:], w2tmp[:], AF.Exp, scale=-0.8)
            w2tp = ps_hb.tile([QT, 128], fp32, tag="hb", name="w2tp")
            te.transpose(w2tp[:], w2e[:], ident[:])
            w2T_bf = constp.tile([QT, 128], bf16, tag="w2T_bf")
            v.tensor_copy(w2T_bf[:], w2tp[:])
            w2b_ps = ps_t.tile([128, QT, 128], fp32, tag="tp", name="w2b_ps")
            for qt in range(QT):
                te.matmul(w2b_ps[:, qt, :],
                          sel_bf[0:QT, qt * 128:(qt + 1) * 128], w2T_bf[:],
                          start=True, stop=True)
            w2b = constp.tile([128, NQ], bf16, tag="w2b")
            sc.copy(w2b[:], w2b_ps[:].rearrange("p a q -> p (a q)"))

            # ---- L. AllGather 2 + panels ----
            if no_cc:
                for r in range(NC):
                    dma.dma_start(ag2_out[r * NQ:(r + 1) * NQ, :], ag2_in[:])
            else:
                g.collective_compute(
                    "AllGather", OP.bypass,
                    ins=[ag2_in.opt()], outs=[ag2_out.opt()],
                    replica_groups=[list(range(NC))],
                )
            pan2 = constp.tile([128, JT, AGC2], fp32, tag="pan2")
            dma.dma_start(pan2[:],
                          ag2_out.rearrange("(t p) c -> p t c", p=128))
            hx2 = constp.tile([128, JT, 17], bf16, tag="hx2")
            sc.copy(hx2[:], pan2[:, :, 0:17])
            b2 = constp.tile([128, JT], fp32, tag="b2")
            sc.activation(b2[:], pan2[:, :, 17], AF.Exp)
            d2 = constp.tile([128, JT], fp32, tag="d2")
            sc.activation(d2[:], pan2[:, :, 17], AF.Exp, scale=ALPHA)
            nb2 = constp.tile([128, JT], fp32, tag="nb2")
            v.tensor_scalar(nb2[:], b2[:], -1.0, None, OP.mult)
            b2_bf = constp.tile([128, JT], bf16, tag="b2_bf")
            v.tensor_copy(b2_bf[:], b2[:])

            # ---- M. layer-2 attention ----
            acc2 = ps_acc.tile([128, QT, 17], fp32, tag="acc", name="acc2")
            hb2_ps = ps_hb.tile([1, 17], fp32, tag="hb", name="hb2_ps")
            n_act2 = 0
            for jt in range(JT):
                e = _SCHED2[jt]
                pt = ppool.tile([128, NQ], bf16, tag="pt", name="pt2")
                if e == 'A':
                    sc.activation(pt[:], w2b[:], AF.Relu,
                                  bias=nb2[:, jt:jt + 1], scale=d2[:, jt:jt + 1])
                    te.matmul(hb2_ps[:], b2_bf[:, jt:jt + 1], hx2[:, jt, :],
                              start=(n_act2 == 0), stop=(jt == 27))
                    n_act2 += 1
                else:
                    eng = v if e == 'D' else g
                    eng.tensor_scalar(pt[:], w2b[:], d2[:, jt:jt + 1],
                                      b2[:, jt:jt + 1], OP.mult, OP.max)
                for qc in range(QT):
                    te.matmul(acc2[:, qc, :], pt[:, qc * 128:(qc + 1) * 128],
                              hx2[:, jt, :], start=(jt == 0), stop=False)
            hb2_sb = workp.tile([1, 17], bf16, tag="hb2_sb")
            v.tensor_copy(hb2_sb[:], hb2_ps[:])
            for qc in range(QT):
                te.matmul(acc2[:, qc, :], ones1[:], hb2_sb[:],
                          start=False, stop=True)

            # ---- N. normalize, elu, log_softmax, store ----
            r2 = workp.tile([128, QT], fp32, tag="r2")
            v.reciprocal(r2[:], acc2[:, :, 16])
            o_all = workp.tile([128, QT, 16], fp32, tag="o_all")
            for qc in range(QT):
                sc.activation(o_all[:, qc, :], acc2[:, qc, 0:16], AF.Copy,
                              scale=r2[:, qc:qc + 1])
            e2 = workp.tile([128, QT, 16], fp32, tag="e2")
            sc.activation(e2[:].rearrange("p a c -> p (a c)"),
                          o_all[:].rearrange("p a c -> p (a c)"), AF.Exp)
            v.tensor_scalar(e2[:].rearrange("p a c -> p (a c)"),
                            e2[:].rearrange("p a c -> p (a c)"),
                            1.0, 0.0, OP.subtract, OP.min)
            v.tensor_tensor(o_all[:].rearrange("p a c -> p (a c)"),
                            o_all[:].rearrange("p a c -> p (a c)"),
                            e2[:].rearrange("p a c -> p (a c)"), OP.max)
            ee = workp.tile([128, QT, 16], fp32, tag="ee")
            sc.activation(ee[:].rearrange("p a c -> p (a c)"),
                          o_all[:].rearrange("p a c -> p (a c)"), AF.Exp)
            s2s = workp.tile([128, QT], fp32, tag="s2s")
            v.tensor_reduce(s2s[:], ee[:], AX, OP.add)
            lse = workp.tile([128, QT], fp32, tag="lse")
            sc.activation(lse[:], s2s[:], AF.Ln)
            fin = workp.tile([128, QT, 16], fp32, tag="fin")
            for qc in range(QT):
                v.tensor_scalar(fin[:, qc, :], o_all[:, qc, :],
                                lse[:, qc:qc + 1], None, OP.subtract)
            dma.dma_start(out.rearrange("(a p) c -> p a c", p=128), fin[:])

    nc.finalize()
    return nc


def _get_compiled(no_cc=False, no_l1=False):
    key = ("nc", no_cc, no_l1)
    if key not in _CACHE:
        _CACHE[key] = _build_nc(no_cc=no_cc, no_l1=no_l1)
    return _CACHE[key]


def kernel(x, Wh, ah, Wo, ao):
    import ml_dtypes
    from concourse.bass_utils import run_bass_kernel_spmd

    bf = ml_dtypes.bfloat16
    nc = _get_compiled()
    x = np.asarray(x, np.float32)
    Wh = np.asarray(Wh, np.float32)
    ah = np.asarray(ah, np.float32)
    Wo = np.asarray(Wo, np.float32)
    ao = np.asarray(ao, np.float32)

    # host-side relayouts (no math): head-major weight matrix, its transpose,
    # block-diag score matrix, split ao
    Whr = np.ascontiguousarray(
        Wh.transpose(1, 0, 2).reshape(NFEAT, HW))          # [512, 512]
    WhrT = np.ascontiguousarray(Whr.T)
    Asd = np.zeros((HW, 16), np.float32)
    for h in range(NHEADS):
        Asd[h * NHID:(h + 1) * NHID, h] = ah[h, :NHID]      # src
        Asd[h * NHID:(h + 1) * NHID, 8 + h] = ah[h, NHID:]  # dst
    aod = np.stack([ao[:NCLASS], ao[NCLASS:]])              # [2, 16]

    Whr_b = Whr.astype(bf)
    WhrT_b = WhrT.astype(bf)
    Asd_b = Asd.astype(bf)
    Wo_b = np.ascontiguousarray(Wo).astype(bf)

    in_maps = []
    for i in range(NC):
        in_maps.append({
            "xT": np.ascontiguousarray(x[i * NQ:(i + 1) * NQ].T).astype(bf),
            "Whr": Whr_b, "WhrT": WhrT_b, "Asd": Asd_b,
            "Wo": Wo_b, "aod": aod,
        })
    res = run_bass_kernel_spmd(nc, in_maps, list(range(NC)))
    return np.concatenate([res.results[i]["out"] for i in range(NC)], 0)


# revision 11
# speedup vs baseline: 1.3899x; 1.0560x over previous
"""GAT (2-layer, 8-head) fused Bass kernel for 8 trn2 NeuronCores.

Sharding: nodes (rows of x) split 512/core. Per core: h computed key-major
with fused score columns; h (bf16) + s_dst (fp32) AllGather'd; each core
computes its 512xN attention block for all 8 heads; layer-1 output projected
and AllGather'd (18 fp32 cols); each core computes its 512xN layer-2 block
and the final log_softmax rows.

Key algebra: with s_i = h_i . a_src, d_j = h_j . a_dst,
  exp(leakyrelu(s_i + d_j)) = max(exp(s_i)exp(d_j), exp(.2 s_i)exp(.2 d_j))
and softmax over j is invariant to any per-i scale, so the attention
numerator is P[j,i] = max(b_j, w_i * dd_j) with b_j = exp(d_j),
w_i = exp(-0.8 s_i), dd_j = exp(0.2 d_j).

P tiles [128 keys, 512 queries] are produced on three engines:
  DVE/Pool: tensor_scalar (mult, max) -> P
  ACT:      relu(dd_j * w_i - b_j) = P - b_j, single activation op; the
            missing rank-1 term hb[c] = sum_j b_j hx[j,c] over ACT-tiles is
            added back into the PSUM accumulation via two tiny matmuls.
Attention matmuls run with the P chunk [128k x 128q] as the *stationary*
operand and the per-head hx block [128, 65] (64 h cols + ones) as the
*moving* operand: 65 columns/matmul instead of 512 -> ~2x less PE time,
and the output lands query-major so normalize/elu/log_softmax use cheap
per-partition scalars.
"""

import numpy as np

N, NFEAT, NHID, NCLASS, NHEADS = 4096, 512, 64, 16, 8
NC = 8                      # cores
NQ = N // NC                # 512 own nodes per core
QT = NQ // 128              # 4 query tiles per core
JT = N // 128               # 32 key tiles
NCH = JT // NC              # 4 key tiles per AG chunk
ALPHA = 0.2
HW = NHID * NHEADS          # 512
HXC = NHEADS * (NHID + 1)   # 520: per-head 64 h cols + ones col
AGC2 = 18                   # AG2: 16 outh + 1 ones + 1 sdst2

# engine schedule for the 32 P-tiles of each layer-1 head sweep
_SCHED1 = ['D'] * JT
for _p in (2, 7, 13, 18, 24, 29):
    _SCHED1[_p] = 'A'
for _p in (4, 10, 16, 21, 27):
    _SCHED1[_p] = 'P'
# layer-2: 32 tiles
_SCHED2 = ['D'] * JT
for _p in (3, 11, 19, 27):
    _SCHED2[_p] = 'A'
for _p in (6, 14, 22, 29):
    _SCHED2[_p] = 'P'

_CACHE = {}


def _build_nc(no_cc=False, no_l1=False):
    import concourse.bass as bass
    import concourse.bacc as bacc
    import concourse.mybir as mybir
    import concourse.tile as tile
    from concourse.masks import make_identity

    fp32 = mybir.dt.float32
    bf16 = mybir.dt.bfloat16
    AX = mybir.AxisListType.X
    OP = mybir.AluOpType
    AF = mybir.ActivationFunctionType

    nc = bacc.Bacc()
    xT = nc.declare_dram_parameter("xT", [NFEAT, NQ], bf16, isOutput=False)
    Whr = nc.declare_dram_parameter("Whr", [NFEAT, HW], bf16, isOutput=False)
    WhrT = nc.declare_dram_parameter("WhrT", [HW, NFEAT], bf16, isOutput=False)
    Asd = nc.declare_dram_parameter("Asd", [HW, 16], bf16, isOutput=False)
    Wo = nc.declare_dram_parameter("Wo", [HW, NCLASS], bf16, isOutput=False)
    aod = nc.declare_dram_parameter("aod", [2, NCLASS], fp32, isOutput=False)
    out = nc.declare_dram_parameter("out", [NQ, NCLASS], fp32, isOutput=True)

    with tile.TileContext(nc) as tc:
        with (
            tc.tile_pool(name="const", bufs=1) as constp,
            tc.tile_pool(name="big", bufs=1) as bigp,
            tc.tile_pool(name="work", bufs=3) as workp,
            tc.tile_pool(name="pp", bufs=20) as ppool,
            tc.tile_pool(name="ps_acc", bufs=3, space="PSUM") as ps_acc,
            tc.tile_pool(name="ps_t", bufs=3, space="PSUM") as ps_t,
            tc.tile_pool(name="ps_hb", bufs=2, space="PSUM") as ps_hb,
            tc.tile_pool(name="dram", bufs=1, space="DRAM") as dramp,
        ):
            v, sc, g, te, dma = nc.vector, nc.scalar, nc.gpsimd, nc.tensor, nc.sync

            ident = constp.tile([128, 128], fp32, tag="ident")
            make_identity(nc, ident[:])
            ident_bf = constp.tile([128, 128], bf16, tag="ident_bf")
            v.tensor_copy(ident_bf[:], ident[:])
            ones1 = constp.tile([1, 128], bf16, tag="ones1")
            g.memset(ones1[:], 1.0)
            # sel[k, h*128+m] = 1 iff k == h (partition-broadcast matmuls)
            self_f = constp.tile([8, 8 * 128], fp32, tag="self_f")
            g.memset(self_f[:], 0.0)
            g.affine_select(
                out=self_f[:].rearrange("k (h m) -> k h m", m=128),
                in_=self_f[:].rearrange("k (h m) -> k h m", m=128),
                compare_op=mybir.AluOpType.not_equal,
                fill=1.0, base=0, channel_multiplier=1,
                pattern=[[-1, 8], [0, 128]])
            sel_bf = constp.tile([8, 8 * 128], bf16, tag="sel_bf")
            sc.copy(sel_bf[:], self_f[:])

            # ---- A. param loads ----
            whrT_sb = constp.tile([128, 4, NFEAT], bf16, tag="whrT_sb")
            dma.dma_start(whrT_sb[:], WhrT.rearrange("(k p) f -> p k f", p=128))
            asd_sb = constp.tile([128, 4, 16], bf16, tag="asd_sb")
            dma.dma_start(asd_sb[:], Asd.rearrange("(k p) s -> p k s", p=128))
            xT_sb = constp.tile([128, 4, NQ], bf16, tag="xT_sb")
            dma.dma_start(xT_sb[:], xT.rearrange("(k p) q -> p k q", p=128))
            whr_sb = constp.tile([128, 4, HW], bf16, tag="whr_sb")
            dma.dma_start(whr_sb[:], Whr.rearrange("(k p) c -> p k c", p=128))
            wo_sb = constp.tile([128, 4, 16], bf16, tag="wo_sb")
            dma.dma_start(wo_sb[:], Wo.rearrange("(k p) s -> p k s", p=128))
            aos_b = constp.tile([128, 16], fp32, tag="aos_b")
            dma.dma_start(aos_b[:], aod[0:1, :].to_broadcast((128, 16)))
            aod_b = constp.tile([128, 16], fp32, tag="aod_b")
            dma.dma_start(aod_b[:], aod[1:2, :].to_broadcast((128, 16)))

            ag1h_in = dramp.tile([NQ, HXC], bf16, tag="ag1h_in")
            ag1h_out = dramp.tile([N, HXC], bf16, tag="ag1h_out",
                                  addr_space="Local" if no_cc else "Shared")
            ag1s_in = dramp.tile([NQ, 8], fp32, tag="ag1s_in")
            ag1s_out = dramp.tile([N, 8], fp32, tag="ag1s_out",
                                  addr_space="Local" if no_cc else "Shared")
            ag2_in = dramp.tile([NQ, AGC2], fp32, tag="ag2_in")
            ag2_out = dramp.tile([N, AGC2], fp32, tag="ag2_out",
                                 addr_space="Local" if no_cc else "Shared")

            # ---- B. Wa_feat = Whr @ Asd  (score weight cols in x-basis) ----
            wa_ps = ps_t.tile([16, NFEAT], fp32, tag="tp", name="wa_ps")
            for k in range(4):
                te.matmul(wa_ps[:], asd_sb[:, k, :], whrT_sb[:, k, :],
                          start=(k == 0), stop=(k == 3))
            waT_sb = constp.tile([16, NFEAT], bf16, tag="waT_sb")
            sc.copy(waT_sb[:], wa_ps[:])
            waf_ps = ps_t.tile([128, 4, 16], bf16, tag="tp", name="waf_ps")
            for k in range(4):
                te.transpose(waf_ps[:, k, :], waT_sb[:, k * 128:(k + 1) * 128],
                             ident_bf[0:16, 0:16])
            wa_f = constp.tile([128, 4, 16], bf16, tag="wa_f")
            v.tensor_copy(wa_f[:], waf_ps[:])

            # ---- C1. s_own for all query tiles; stage + AllGather s ----
            stgs = [bigp.tile([128, 16], fp32, tag=f"stgs{qt}",
                              name=f"stgs{qt}") for qt in range(QT)]
            for qt in range(QT):
                s_ps = ps_t.tile([128, 16], fp32, tag="tp", name="s_ps")
                for k in range(4):
                    te.matmul(s_ps[:], xT_sb[:, k, qt * 128:(qt + 1) * 128],
                              wa_f[:, k, :], start=(k == 0), stop=(k == 3))
                v.tensor_copy(stgs[qt][:], s_ps[:])
                dma.dma_start(ag1s_in[qt * 128:(qt + 1) * 128, :],
                              stgs[qt][:, 8:16])
            if no_cc:
                for r in range(NC):
                    dma.dma_start(ag1s_out[r * NQ:(r + 1) * NQ, :], ag1s_in[:])
            else:
                g.collective_compute(
                    "AllGather", OP.bypass,
                    ins=[ag1s_in.opt()], outs=[ag1s_out.opt()],
                    replica_groups=[list(range(NC))],
                )
            sd_pan = constp.tile([128, JT * 8], fp32, tag="sd_pan")
            dma.dma_start(
                sd_pan[:].rearrange("p (t h) -> p t h", h=8),
                ag1s_out.rearrange("(t p) h -> p t h", p=128))

            # ---- C2. h_own key-major; stage + AllGather h ----
            stg = [bigp.tile([128, HXC], bf16, tag=f"stg{qt}", name=f"stg{qt}")
                   for qt in range(QT)]
            for qt in range(QT):
                h_ps = ps_acc.tile([128, HW], fp32, tag="acc", name="h_ps")
                for k in range(4):
                    te.matmul(h_ps[:], xT_sb[:, k, qt * 128:(qt + 1) * 128],
                              whr_sb[:, k, :], start=(k == 0), stop=(k == 3))
                eng = sc if qt % 2 else v
                eng_c = eng.copy if qt % 2 else eng.tensor_copy
                eng_c(stg[qt][:].rearrange("p (h c) -> p h c", c=65)[:, :, 0:64],
                      h_ps[:].rearrange("p (h c) -> p h c", c=64))
                g.memset(
                    stg[qt][:].rearrange("p (h c) -> p h c", c=65)[:, :, 64:65],
                    1.0)
                dma.dma_start(ag1h_in[qt * 128:(qt + 1) * 128, :], stg[qt][:])
            if no_cc:
                for r in range(NC):
                    dma.dma_start(ag1h_out[r * NQ:(r + 1) * NQ, :], ag1h_in[:])
            else:
                g.collective_compute(
                    "AllGather", OP.bypass,
                    ins=[ag1h_in.opt()], outs=[ag1h_out.opt()],
                    replica_groups=[list(range(NC))],
                )

            # ---- D. w panel (own s_src): transpose, exp, broadcast ----
            s_fm = ps_t.tile([16, NQ], fp32, tag="tp", name="s_fm")
            for qt in range(QT):
                te.transpose(s_fm[:, qt * 128:(qt + 1) * 128], stgs[qt][:],
                             ident[0:128, 0:128])
            w_bf = constp.tile([8, NQ], bf16, tag="w_bf")
            sc.activation(w_bf[:], s_fm[0:8, :], AF.Exp, scale=-0.8)
            wb_all = constp.tile([128, NHEADS, NQ], bf16, tag="wb_all")
            for h in range(NHEADS):
                wb_ps = ps_t.tile([128, NQ], fp32, tag="tp", name="wb_ps")
                te.matmul(wb_ps[:], sel_bf[:, h * 128:(h + 1) * 128], w_bf[:],
                          start=True, stop=True)
                eng = (v.tensor_copy, sc.copy, sc.copy)[h % 3]
                eng(wb_all[:, h, :], wb_ps[:])

            # ---- F. key-side score panels ----
            b_all = constp.tile([128, JT * 8], fp32, tag="b_all")
            sc.activation(b_all[:], sd_pan[:], AF.Exp)
            d_all = constp.tile([128, JT * 8], fp32, tag="d_all")
            sc.activation(d_all[:], sd_pan[:], AF.Exp, scale=ALPHA)
            nb_all = constp.tile([128, JT * 8], fp32, tag="nb_all")
            v.tensor_scalar(nb_all[:], b_all[:], -1.0, None, OP.mult)
            b_bf = constp.tile([128, JT * 8], bf16, tag="b_bf")
            v.tensor_copy(b_bf[:], b_all[:])

            # ---- G. hx chunk loads (gated per AG chunk) ----
            hx = []
            for c in range(NC):
                t = bigp.tile([128, NCH, HXC], bf16, tag=f"hx{c}",
                              name=f"hx{c}")
                dma.dma_start(
                    t[:],
                    ag1h_out[c * NQ:(c + 1) * NQ, :].rearrange(
                        "(a p) x -> p a x", p=128))
                hx.append(t)

            def hx_slice(jt, h, w):
                return hx[jt // NCH][:, jt % NCH, h * 65:h * 65 + w]

            # ---- H. layer-1 attention ----
            xc_pre = [bigp.tile([128, HW], fp32, tag=f"xc{qc}",
                                name=f"xc{qc}") for qc in range(QT)]
            r_pan = constp.tile([128, NHEADS, QT], fp32, tag="r_pan")

            for h in range(NHEADS if not no_l1 else 0):
                acc = ps_acc.tile([128, QT, 65], fp32, tag="acc", name="acc")
                act_jts = []
                hb_ps = ps_hb.tile([1, 65], fp32, tag="hb", name="hb_ps")
                for idx in range(JT):
                    jt = (h * NCH + idx) % JT
                    col = slice(jt * 8 + h, jt * 8 + h + 1)
                    e = _SCHED1[idx]
                    pt = ppool.tile([128, NQ], bf16, tag="pt", name="pt")
                    if e == 'A':
                        sc.activation(pt[:], wb_all[:, h, :], AF.Relu,
                                      bias=nb_all[:, col], scale=d_all[:, col])
                        te.matmul(hb_ps[:], b_bf[:, col], hx_slice(jt, h, 65),
                                  start=(not act_jts), stop=(idx == 29))
                        act_jts.append(jt)
                    else:
                        eng = v if e == 'D' else g
                        eng.tensor_scalar(pt[:], wb_all[:, h, :],
                                          d_all[:, col], b_all[:, col],
                                          OP.mult, OP.max)
                    for qc in range(QT):
                        te.matmul(acc[:, qc, :],
                                  pt[:, qc * 128:(qc + 1) * 128],
                                  hx_slice(jt, h, 65),
                                  start=(idx == 0), stop=False)
                # rank-1 correction for the ACT-produced tiles: acc += 1 (x) hb
                hb_sb = workp.tile([1, 65], bf16, tag="hb_sb", bufs=2)
                v.tensor_copy(hb_sb[:], hb_ps[:])
                for qc in range(QT):
                    te.matmul(acc[:, qc, :], ones1[:], hb_sb[:],
                              start=False, stop=True)
                # normalize: r = 1/den, xc_pre[:, h*64:] = f * r
                v.reciprocal(r_pan[:, h, :], acc[:, :, 64])
                for qc in range(QT):
                    dst = xc_pre[qc][:, h * 64:(h + 1) * 64]
                    if (h * QT + qc) % 2:
                        sc.activation(dst, acc[:, qc, 0:64], AF.Copy,
                                      scale=r_pan[:, h, qc:qc + 1])
                    else:
                        v.tensor_scalar(dst, acc[:, qc, 0:64],
                                        r_pan[:, h, qc:qc + 1], None, OP.mult)

            if no_l1:
                for qc in range(QT):
                    g.memset(xc_pre[qc][:], 0.5)

            # ---- I/J. per-query-tile: elu (fp32), transpose, project,
            # stage AG2 ----
            w2tmp = constp.tile([128, QT], fp32, tag="w2tmp")
            for qc in range(QT):
                e1 = workp.tile([128, HW], fp32, tag="elu_e", name="e1")
                sc.activation(e1[:], xc_pre[qc][:], AF.Exp)
                v.tensor_scalar(e1[:], e1[:], 1.0, 0.0, OP.subtract, OP.min)
                v.tensor_tensor(xc_pre[qc][:], xc_pre[qc][:], e1[:], OP.max)
                tp = ps_t.tile([128, 4, 128], fp32, tag="tp", name="tp_xc")
                for fc in range(4):
                    te.transpose(tp[:, fc, :],
                                 xc_pre[qc][:, fc * 128:(fc + 1) * 128],
                                 ident[:])
                xcT = bigp.tile([128, 4, 128], bf16, tag=f"xcT{qc}",
                                name=f"xcT{qc}")
                (sc.copy if qc % 2 else v.tensor_copy)(xcT[:], tp[:])
                o_ps = ps_t.tile([128, 16], fp32, tag="tp", name="o_ps")
                for fc in range(4):
                    te.matmul(o_ps[:], xcT[:, fc, :], wo_sb[:, fc, :],
                              start=(fc == 0), stop=(fc == 3))
                stg2 = bigp.tile([128, AGC2], fp32, tag=f"stg2_{qc}",
                                 name=f"stg2_{qc}")
                v.tensor_copy(stg2[:, 0:16], o_ps[:])
                g.memset(stg2[:, 16:17], 1.0)
                tmp = workp.tile([128, 16], fp32, tag="sdtmp")
                v.scalar_tensor_tensor(tmp[:], o_ps[:], 1.0, aod_b[:],
                                       OP.mult, OP.mult,
                                       accum_out=stg2[:, 17:18])
                tmp2 = workp.tile([128, 16], fp32, tag="sdtmp2")
                v.scalar_tensor_tensor(tmp2[:], o_ps[:], 1.0, aos_b[:],
                                       OP.mult, OP.mult,
                                       accum_out=w2tmp[:, qc:qc + 1])
                dma.dma_start(ag2_in[qc * 128:(qc + 1) * 128, :], stg2[:])

            # ---- K. w2 broadcast panel ----
            w2e = constp.tile([128, QT], fp32, tag="w2e")
            sc.activation(w2e[:], w2tmp[:], AF.Exp, scale=-0.8)
            w2tp = ps_hb.tile([QT, 128], fp32, tag="hb", name="w2tp")
            te.transpose(w2tp[:], w2e[:], ident[:])
            w2T_bf = constp.tile([QT, 128], bf16, tag="w2T_bf")
            v.tensor_copy(w2T_bf[:], w2tp[:])
            w2b_ps = ps_t.tile([128, QT, 128], fp32, tag="tp", name="w2b_ps")
            for qt in range(QT):
                te.matmul(w2b_ps[:, qt, :],
                          sel_bf[0:QT, qt * 128:(qt + 1) * 128], w2T_bf[:],
                          start=True, stop=True)
            w2b = constp.tile([128, NQ], bf16, tag="w2b")
            sc.copy(w2b[:], w2b_ps[:].rearrange("p a q -> p (a q)"))

            # ---- L. AllGather 2 + panels (two halves for earlier start) ----
            if no_cc:
                for r in range(NC):
                    dma.dma_start(ag2_out[r * NQ:(r + 1) * NQ, :], ag2_in[:])
            else:
                g.collective_compute(
                    "AllGather", OP.bypass,
                    ins=[ag2_in.opt()], outs=[ag2_out.opt()],
                    replica_groups=[list(range(NC))],
                )
            pan2 = constp.tile([128, JT, AGC2], fp32, tag="pan2")
            hx2 = constp.tile([128, JT, 17], bf16, tag="hx2")
            b2 = constp.tile([128, JT], fp32, tag="b2")
            d2 = constp.tile([128, JT], fp32, tag="d2")
            nb2 = constp.tile([128, JT], fp32, tag="nb2")
            b2_bf = constp.tile([128, JT], bf16, tag="b2_bf")
            HJ = JT // 2
            for hf in range(2):
                js = slice(hf * HJ, (hf + 1) * HJ)
                dma.dma_start(
                    pan2[:, js, :],
                    ag2_out[hf * (N // 2):(hf + 1) * (N // 2), :].rearrange(
                        "(t p) c -> p t c", p=128))
                sc.copy(hx2[:, js, :], pan2[:, js, 0:17])
                sc.activation(b2[:, js], pan2[:, js, 17], AF.Exp)
                sc.activation(d2[:, js], pan2[:, js, 17], AF.Exp, scale=ALPHA)
                v.tensor_scalar(nb2[:, js], b2[:, js], -1.0, None, OP.mult)
                v.tensor_copy(b2_bf[:, js], b2[:, js])

            # ---- M. layer-2 attention ----
            acc2 = ps_acc.tile([128, QT, 17], fp32, tag="acc", name="acc2")
            hb2_ps = ps_hb.tile([1, 17], fp32, tag="hb", name="hb2_ps")
            n_act2 = 0
            for jt in range(JT):
                e = _SCHED2[jt]
                pt = ppool.tile([128, NQ], bf16, tag="pt", name="pt2")
                if e == 'A':
                    sc.activation(pt[:], w2b[:], AF.Relu,
                                  bias=nb2[:, jt:jt + 1], scale=d2[:, jt:jt + 1])
                    te.matmul(hb2_ps[:], b2_bf[:, jt:jt + 1], hx2[:, jt, :],
                              start=(n_act2 == 0), stop=(jt == 27))
                    n_act2 += 1
                else:
                    eng = v if e == 'D' else g
                    eng.tensor_scalar(pt[:], w2b[:], d2[:, jt:jt + 1],
                                      b2[:, jt:jt + 1], OP.mult, OP.max)
                for qc in range(QT):
                    te.matmul(acc2[:, qc, :], pt[:, qc * 128:(qc + 1) * 128],
                              hx2[:, jt, :], start=(jt == 0), stop=False)
            hb2_sb = workp.tile([1, 17], bf16, tag="hb2_sb")
            v.tensor_copy(hb2_sb[:], hb2_ps[:])
            for qc in range(QT):
                te.matmul(acc2[:, qc, :], ones1[:], hb2_sb[:],
                          start=False, stop=True)

            # ---- N. normalize, elu, log_softmax, store ----
            r2 = workp.tile([128, QT], fp32, tag="r2")
            v.reciprocal(r2[:], acc2[:, :, 16])
            o_all = workp.tile([128, QT, 16], fp32, tag="o_all")
            for qc in range(QT):
                sc.activation(o_all[:, qc, :], acc2[:, qc, 0:16], AF.Copy,
                              scale=r2[:, qc:qc + 1])
            e2 = workp.tile([128, QT, 16], fp32, tag="e2")
            sc.activation(e2[:].rearrange("p a c -> p (a c)"),
                          o_all[:].rearrange("p a c -> p (a c)"), AF.Exp)
            v.tensor_scalar(e2[:].rearrange("p a c -> p (a c)"),
                            e2[:].rearrange("p a c -> p (a c)"),
                            1.0, 0.0, OP.subtract, OP.min)
            v.tensor_tensor(o_all[:].rearrange("p a c -> p (a c)"),
                            o_all[:].rearrange("p a c -> p (a c)"),
                            e2[:].rearrange("p a c -> p (a c)"), OP.max)
            ee = workp.tile([128, QT, 16], fp32, tag="ee")
            sc.activation(ee[:].rearrange("p a c -> p (a c)"),
                          o_all[:].rearrange("p a c -> p (a c)"), AF.Exp)
            s2s = workp.tile([128, QT], fp32, tag="s2s")
            v.tensor_reduce(s2s[:], ee[:], AX, OP.add)
            lse = workp.tile([128, QT], fp32, tag="lse")
            sc.activation(lse[:], s2s[:], AF.Ln)
            fin = workp.tile([128, QT, 16], fp32, tag="fin")
            for qc in range(QT):
                v.tensor_scalar(fin[:, qc, :], o_all[:, qc, :],
                                lse[:, qc:qc + 1], None, OP.subtract)
            dma.dma_start(out.rearrange("(a p) c -> p a c", p=128), fin[:])

    nc.finalize()
    return nc


def _get_compiled(no_cc=False, no_l1=False):
    key = ("nc", no_cc, no_l1)
    if key not in _CACHE:
        _CACHE[key] = _build_nc(no_cc=no_cc, no_l1=no_l1)
    return _CACHE[key]


def kernel(x, Wh, ah, Wo, ao):
    import ml_dtypes
    from concourse.bass_utils import run_bass_kernel_spmd

    bf = ml_dtypes.bfloat16
    nc = _get_compiled()
    x = np.asarray(x, np.float32)
    Wh = np.asarray(Wh, np.float32)
    ah = np.asarray(ah, np.float32)
    Wo = np.asarray(Wo, np.float32)
    ao = np.asarray(ao, np.float32)

    # host-side relayouts (no math): head-major weight matrix, its transpose,
    # block-diag score matrix, split ao
    Whr = np.ascontiguousarray(
        Wh.transpose(1, 0, 2).reshape(NFEAT, HW))          # [512, 512]
    WhrT = np.ascontiguousarray(Whr.T)
    Asd = np.zeros((HW, 16), np.float32)
    for h in range(NHEADS):
        Asd[h * NHID:(h + 1) * NHID, h] = ah[h, :NHID]      # src
        Asd[h * NHID:(h + 1) * NHID, 8 + h] = ah[h, NHID:]  # dst
    aod = np.stack([ao[:NCLASS], ao[NCLASS:]])              # [2, 16]

    Whr_b = Whr.astype(bf)
    WhrT_b = WhrT.astype(bf)
    Asd_b = Asd.astype(bf)
    Wo_b = np.ascontiguousarray(Wo).astype(bf)

    in_maps = []
    for i in range(NC):
        in_maps.append({
            "xT": np.ascontiguousarray(x[i * NQ:(i + 1) * NQ].T).astype(bf),
            "Whr": Whr_b, "WhrT": WhrT_b, "Asd": Asd_b,
            "Wo": Wo_b, "aod": aod,
        })
    res = run_bass_kernel_spmd(nc, in_maps, list(range(NC)))
    return np.concatenate([res.results[i]["out"] for i in range(NC)], 0)


# revision 16
# speedup vs baseline: 1.4444x; 1.0393x over previous
"""GAT (2-layer, 8-head) fused Bass kernel for 8 trn2 NeuronCores.

Sharding: nodes (rows of x) split 512/core. Per core: h computed key-major
with fused score columns; h (bf16) + s_dst (fp32) AllGather'd; each core
computes its 512xN attention block for all 8 heads; layer-1 output projected
and AllGather'd (18 fp32 cols); each core computes its 512xN layer-2 block
and the final log_softmax rows.

Key algebra: with s_i = h_i . a_src, d_j = h_j . a_dst,
  exp(leakyrelu(s_i + d_j)) = max(exp(s_i)exp(d_j), exp(.2 s_i)exp(.2 d_j))
and softmax over j is invariant to any per-i scale, so the attention
numerator is P[j,i] = max(b_j, w_i * dd_j) with b_j = exp(d_j),
w_i = exp(-0.8 s_i), dd_j = exp(0.2 d_j).

P tiles [128 keys, 512 queries] are produced on three engines:
  DVE/Pool: tensor_scalar (mult, max) -> P
  ACT:      relu(dd_j * w_i - b_j) = P - b_j, single activation op; the
            missing rank-1 term hb[c] = sum_j b_j hx[j,c] over ACT-tiles is
            added back into the PSUM accumulation via two tiny matmuls.
Attention matmuls run with the P chunk [128k x 128q] as the *stationary*
operand and the per-head hx block [128, 65] (64 h cols + ones) as the
*moving* operand: 65 columns/matmul instead of 512 -> ~2x less PE time,
and the output lands query-major so normalize/elu/log_softmax use cheap
per-partition scalars.
"""

import numpy as np

N, NFEAT, NHID, NCLASS, NHEADS = 4096, 512, 64, 16, 8
NC = 8                      # cores
NQ = N // NC                # 512 own nodes per core
QT = NQ // 128              # 4 query tiles per core
JT = N // 128               # 32 key tiles
NCH = JT // NC              # 4 key tiles per AG chunk
ALPHA = 0.2
HW = NHID * NHEADS          # 512
HXC = NHEADS * (NHID + 1)   # 520: per-head 64 h cols + ones col
AGC2 = 18                   # AG2: 16 outh + 1 ones + 1 sdst2

# engine schedule for the 32 P-tiles of each layer-1 head sweep
_SCHED1 = ['D'] * JT
for _p in (2, 7, 13, 18, 24, 29):
    _SCHED1[_p] = 'A'
for _p in (4, 10, 16, 21, 27):
    _SCHED1[_p] = 'P'
# layer-2: 32 tiles
_SCHED2 = ['D'] * JT
for _p in (3, 11, 19, 27):
    _SCHED2[_p] = 'A'
for _p in (6, 14, 22, 29):
    _SCHED2[_p] = 'P'

_CACHE = {}


def _build_nc(no_cc=False, no_l1=False):
    import concourse.bass as bass
    import concourse.bacc as bacc
    import concourse.mybir as mybir
    import concourse.tile as tile
    from concourse.masks import make_identity

    fp32 = mybir.dt.float32
    bf16 = mybir.dt.bfloat16
    AX = mybir.AxisListType.X
    OP = mybir.AluOpType
    AF = mybir.ActivationFunctionType

    nc = bacc.Bacc()
    xT = nc.declare_dram_parameter("xT", [NFEAT, NQ], bf16, isOutput=False)
    Whr = nc.declare_dram_parameter("Whr", [NFEAT, HW], bf16, isOutput=False)
    WhrT = nc.declare_dram_parameter("WhrT", [HW, NFEAT], bf16, isOutput=False)
    Asd = nc.declare_dram_parameter("Asd", [HW, 16], bf16, isOutput=False)
    Wo = nc.declare_dram_parameter("Wo", [HW, NCLASS], bf16, isOutput=False)
    aod = nc.declare_dram_parameter("aod", [2, NCLASS], fp32, isOutput=False)
    out = nc.declare_dram_parameter("out", [NQ, NCLASS], fp32, isOutput=True)

    with tile.TileContext(nc) as tc:
        with (
            tc.tile_pool(name="const", bufs=1) as constp,
            tc.tile_pool(name="big", bufs=1) as bigp,
            tc.tile_pool(name="work", bufs=3) as workp,
            tc.tile_pool(name="pp", bufs=44) as ppool,
            tc.tile_pool(name="ps_acc", bufs=3, space="PSUM") as ps_acc,
            tc.tile_pool(name="ps_t", bufs=3, space="PSUM") as ps_t,
            tc.tile_pool(name="ps_hb", bufs=2, space="PSUM") as ps_hb,
            tc.tile_pool(name="dram", bufs=1, space="DRAM") as dramp,
        ):
            v, sc, g, te, dma = nc.vector, nc.scalar, nc.gpsimd, nc.tensor, nc.sync

            ident = constp.tile([128, 128], fp32, tag="ident")
            make_identity(nc, ident[:])
            ident_bf = constp.tile([128, 128], bf16, tag="ident_bf")
            v.tensor_copy(ident_bf[:], ident[:])
            ones1 = constp.tile([1, 128], bf16, tag="ones1")
            g.memset(ones1[:], 1.0)
            # sel[k, h*128+m] = 1 iff k == h (partition-broadcast matmuls)
            self_f = constp.tile([8, 8 * 128], fp32, tag="self_f")
            g.memset(self_f[:], 0.0)
            g.affine_select(
                out=self_f[:].rearrange("k (h m) -> k h m", m=128),
                in_=self_f[:].rearrange("k (h m) -> k h m", m=128),
                compare_op=mybir.AluOpType.not_equal,
                fill=1.0, base=0, channel_multiplier=1,
                pattern=[[-1, 8], [0, 128]])
            sel_bf = constp.tile([8, 8 * 128], bf16, tag="sel_bf")
            sc.copy(sel_bf[:], self_f[:])

            # ---- A. param loads (xT/Whr first: h matmuls are the critical
            # path to the AG1 stream) ----
            xT_sb = constp.tile([128, 4, NQ], bf16, tag="xT_sb")
            dma.dma_start(xT_sb[:], xT.rearrange("(k p) q -> p k q", p=128))
            whr_sb = constp.tile([128, 4, HW], bf16, tag="whr_sb")
            dma.dma_start(whr_sb[:], Whr.rearrange("(k p) c -> p k c", p=128))
            whrT_sb = constp.tile([128, 4, NFEAT], bf16, tag="whrT_sb")
            dma.dma_start(whrT_sb[:], WhrT.rearrange("(k p) f -> p k f", p=128))
            asd_sb = constp.tile([128, 4, 16], bf16, tag="asd_sb")
            dma.dma_start(asd_sb[:], Asd.rearrange("(k p) s -> p k s", p=128))
            wo_sb = constp.tile([128, 4, 16], bf16, tag="wo_sb")
            dma.dma_start(wo_sb[:], Wo.rearrange("(k p) s -> p k s", p=128))
            aos_b = constp.tile([128, 16], fp32, tag="aos_b")
            dma.dma_start(aos_b[:], aod[0:1, :].to_broadcast((128, 16)))
            aod_b = constp.tile([128, 16], fp32, tag="aod_b")
            dma.dma_start(aod_b[:], aod[1:2, :].to_broadcast((128, 16)))

            ag1h_in = dramp.tile([NQ, HXC], bf16, tag="ag1h_in")
            ag1h_out = dramp.tile([N, HXC], bf16, tag="ag1h_out",
                                  addr_space="Local" if no_cc else "Shared")
            ag1s_in = dramp.tile([NQ, 8], fp32, tag="ag1s_in")
            ag1s_out = dramp.tile([N, 8], fp32, tag="ag1s_out",
                                  addr_space="Local" if no_cc else "Shared")
            ag2_in = dramp.tile([NQ, AGC2], fp32, tag="ag2_in")
            ag2_out = dramp.tile([N, AGC2], fp32, tag="ag2_out",
                                 addr_space="Local" if no_cc else "Shared")

            # ---- B. h_own key-major; stage + AllGather h.  The h matmul
            # chain runs first (only xT/Whr deps, back-to-back PE work warms
            # the p-state) so the AG1h DMA stream starts as early as
            # possible: it is the long pole feeding the attention sweep. ----
            stg = [bigp.tile([128, HXC], bf16, tag=f"stg{qt}", name=f"stg{qt}")
                   for qt in range(QT)]
            for qt in range(QT):
                h_ps = ps_acc.tile([128, HW], fp32, tag="acc", name="h_ps")
                for k in range(4):
                    te.matmul(h_ps[:], xT_sb[:, k, qt * 128:(qt + 1) * 128],
                              whr_sb[:, k, :], start=(k == 0), stop=(k == 3))
                eng = sc if qt % 2 else v
                eng_c = eng.copy if qt % 2 else eng.tensor_copy
                eng_c(stg[qt][:].rearrange("p (h c) -> p h c", c=65)[:, :, 0:64],
                      h_ps[:].rearrange("p (h c) -> p h c", c=64))
                g.memset(
                    stg[qt][:].rearrange("p (h c) -> p h c", c=65)[:, :, 64:65],
                    1.0)
                dma.dma_start(ag1h_in[qt * 128:(qt + 1) * 128, :], stg[qt][:])

            # ---- C. Wa_feat = Whr @ Asd; s_own; stage + AllGather s ----
            wa_ps = ps_t.tile([16, NFEAT], fp32, tag="tp", name="wa_ps")
            for k in range(4):
                te.matmul(wa_ps[:], asd_sb[:, k, :], whrT_sb[:, k, :],
                          start=(k == 0), stop=(k == 3))
            waT_sb = constp.tile([16, NFEAT], bf16, tag="waT_sb")
            sc.copy(waT_sb[:], wa_ps[:])
            waf_ps = ps_t.tile([128, 4, 16], bf16, tag="tp", name="waf_ps")
            for k in range(4):
                te.transpose(waf_ps[:, k, :], waT_sb[:, k * 128:(k + 1) * 128],
                             ident_bf[0:16, 0:16])
            wa_f = constp.tile([128, 4, 16], bf16, tag="wa_f")
            v.tensor_copy(wa_f[:], waf_ps[:])

            stgs = [bigp.tile([128, 16], fp32, tag=f"stgs{qt}",
                              name=f"stgs{qt}") for qt in range(QT)]
            for qt in range(QT):
                s_ps = ps_t.tile([128, 16], fp32, tag="tp", name="s_ps")
                for k in range(4):
                    te.matmul(s_ps[:], xT_sb[:, k, qt * 128:(qt + 1) * 128],
                              wa_f[:, k, :], start=(k == 0), stop=(k == 3))
                v.tensor_copy(stgs[qt][:], s_ps[:])
                dma.dma_start(ag1s_in[qt * 128:(qt + 1) * 128, :],
                              stgs[qt][:, 8:16])
            if no_cc:
                dma.dma_start(
                    ag1s_out.rearrange("(r q) h -> r q h", r=NC),
                    ag1s_in.rearrange("(o q) h -> o q h", o=1).to_broadcast(
                        (NC, NQ, 8)))
            else:
                g.collective_compute(
                    "AllGather", OP.bypass,
                    ins=[ag1s_in.opt()], outs=[ag1s_out.opt()],
                    replica_groups=[list(range(NC))],
                )
            sd_pan = constp.tile([128, JT * 8], fp32, tag="sd_pan")
            dma.dma_start(
                sd_pan[:].rearrange("p (t h) -> p t h", h=8),
                ag1s_out.rearrange("(t p) h -> p t h", p=128))

            # ---- E. AG1h replica copies interleaved with hx chunk loads ----
            hx = []
            for c in range(NC):
                if no_cc:
                    dma.dma_start(ag1h_out[c * NQ:(c + 1) * NQ, :],
                                  ag1h_in[:])
                elif c == 0:
                    g.collective_compute(
                        "AllGather", OP.bypass,
                        ins=[ag1h_in.opt()], outs=[ag1h_out.opt()],
                        replica_groups=[list(range(NC))],
                    )
                t = bigp.tile([128, NCH, HXC], bf16, tag=f"hx{c}",
                              name=f"hx{c}")
                dma.dma_start(
                    t[:],
                    ag1h_out[c * NQ:(c + 1) * NQ, :].rearrange(
                        "(a p) x -> p a x", p=128))
                hx.append(t)

            # ---- D. w panel (own s_src): transpose, exp, broadcast ----
            s_fm = ps_t.tile([16, NQ], fp32, tag="tp", name="s_fm")
            for qt in range(QT):
                te.transpose(s_fm[:, qt * 128:(qt + 1) * 128], stgs[qt][:],
                             ident[0:128, 0:128])
            w_bf = constp.tile([8, NQ], bf16, tag="w_bf")
            sc.activation(w_bf[:], s_fm[0:8, :], AF.Exp, scale=-0.8)
            wb_all = constp.tile([128, NHEADS, NQ], bf16, tag="wb_all")
            for h in range(NHEADS):
                wb_ps = ps_t.tile([128, NQ], fp32, tag="tp", name="wb_ps")
                te.matmul(wb_ps[:], sel_bf[:, h * 128:(h + 1) * 128], w_bf[:],
                          start=True, stop=True)
                eng = (v.tensor_copy, sc.copy, sc.copy)[h % 3]
                eng(wb_all[:, h, :], wb_ps[:])

            # ---- F. key-side score panels ----
            b_all = constp.tile([128, JT * 8], fp32, tag="b_all")
            sc.activation(b_all[:], sd_pan[:], AF.Exp)
            d_all = constp.tile([128, JT * 8], fp32, tag="d_all")
            sc.activation(d_all[:], sd_pan[:], AF.Exp, scale=ALPHA)
            nb_all = constp.tile([128, JT * 8], fp32, tag="nb_all")
            v.tensor_scalar(nb_all[:], b_all[:], -1.0, None, OP.mult)
            b_bf = constp.tile([128, JT * 8], bf16, tag="b_bf")
            v.tensor_copy(b_bf[:], b_all[:])

            def hx_slice(jt, h, w):
                return hx[jt // NCH][:, jt % NCH, h * 65:h * 65 + w]

            # ---- H. layer-1 attention ----
            xc_pre = [bigp.tile([128, HW], fp32, tag=f"xc{qc}",
                                name=f"xc{qc}") for qc in range(QT)]
            r_pan = constp.tile([128, NHEADS, QT], fp32, tag="r_pan")

            for h in range(NHEADS if not no_l1 else 0):
                acc = ps_acc.tile([128, QT, 65], fp32, tag="acc", name="acc")
                act_jts = []
                hb_ps = ps_hb.tile([1, 65], fp32, tag="hb", name="hb_ps")
                for idx in range(JT):
                    jt = (h * NCH + idx) % JT
                    col = slice(jt * 8 + h, jt * 8 + h + 1)
                    e = _SCHED1[idx]
                    pt = ppool.tile([128, NQ], bf16, tag="pt", name="pt")
                    if e == 'A':
                        sc.activation(pt[:], wb_all[:, h, :], AF.Relu,
                                      bias=nb_all[:, col], scale=d_all[:, col])
                        te.matmul(hb_ps[:], b_bf[:, col], hx_slice(jt, h, 65),
                                  start=(not act_jts), stop=(idx == 29))
                        act_jts.append(jt)
                    else:
                        eng = v if e == 'D' else g
                        eng.tensor_scalar(pt[:], wb_all[:, h, :],
                                          d_all[:, col], b_all[:, col],
                                          OP.mult, OP.max)
                    for qc in range(QT):
                        te.matmul(acc[:, qc, :],
                                  pt[:, qc * 128:(qc + 1) * 128],
                                  hx_slice(jt, h, 65),
                                  start=(idx == 0), stop=False)
                # rank-1 correction for the ACT-produced tiles: acc += 1 (x) hb
                hb_sb = workp.tile([1, 65], bf16, tag="hb_sb", bufs=2)
                v.tensor_copy(hb_sb[:], hb_ps[:])
                for qc in range(QT):
                    te.matmul(acc[:, qc, :], ones1[:], hb_sb[:],
                              start=False, stop=True)
                # normalize: r = 1/den, xc_pre[:, h*64:] = f * r
                v.reciprocal(r_pan[:, h, :], acc[:, :, 64])
                for qc in range(QT):
                    dst = xc_pre[qc][:, h * 64:(h + 1) * 64]
                    if (h * QT + qc) % 2:
                        sc.activation(dst, acc[:, qc, 0:64], AF.Copy,
                                      scale=r_pan[:, h, qc:qc + 1])
                    else:
                        v.tensor_scalar(dst, acc[:, qc, 0:64],
                                        r_pan[:, h, qc:qc + 1], None, OP.mult)

            if no_l1:
                for qc in range(QT):
                    g.memset(xc_pre[qc][:], 0.5)

            # ---- I/J. per-query-tile: elu (fp32), transpose, project,
            # stage AG2 ----
            w2tmp = constp.tile([128, QT], fp32, tag="w2tmp")
            for qc in range(QT):
                e1 = workp.tile([128, HW], fp32, tag="elu_e", name="e1")
                sc.activation(e1[:], xc_pre[qc][:], AF.Exp)
                g.tensor_scalar(e1[:], e1[:], 1.0, 0.0, OP.subtract, OP.min)
                v.tensor_tensor(xc_pre[qc][:], xc_pre[qc][:], e1[:], OP.max)
                tp = ps_t.tile([128, 4, 128], fp32, tag="tp", name="tp_xc")
                for fc in range(4):
                    te.transpose(tp[:, fc, :],
                                 xc_pre[qc][:, fc * 128:(fc + 1) * 128],
                                 ident[:])
                xcT = bigp.tile([128, 4, 128], bf16, tag=f"xcT{qc}",
                                name=f"xcT{qc}")
                (sc.copy if qc % 2 else v.tensor_copy)(xcT[:], tp[:])
                o_ps = ps_t.tile([128, 16], fp32, tag="tp", name="o_ps")
                for fc in range(4):
                    te.matmul(o_ps[:], xcT[:, fc, :], wo_sb[:, fc, :],
                              start=(fc == 0), stop=(fc == 3))
                stg2 = bigp.tile([128, AGC2], fp32, tag=f"stg2_{qc}",
                                 name=f"stg2_{qc}")
                v.tensor_copy(stg2[:, 0:16], o_ps[:])
                g.memset(stg2[:, 16:17], 1.0)
                tmp = workp.tile([128, 16], fp32, tag="sdtmp")
                v.scalar_tensor_tensor(tmp[:], o_ps[:], 1.0, aod_b[:],
                                       OP.mult, OP.mult,
                                       accum_out=stg2[:, 17:18])
                tmp2 = workp.tile([128, 16], fp32, tag="sdtmp2")
                v.scalar_tensor_tensor(tmp2[:], o_ps[:], 1.0, aos_b[:],
                                       OP.mult, OP.mult,
                                       accum_out=w2tmp[:, qc:qc + 1])
                dma.dma_start(ag2_in[qc * 128:(qc + 1) * 128, :], stg2[:])

            # ---- K. w2 broadcast panel ----
            w2e = constp.tile([128, QT], fp32, tag="w2e")
            sc.activation(w2e[:], w2tmp[:], AF.Exp, scale=-0.8)
            w2tp = ps_hb.tile([QT, 128], fp32, tag="hb", name="w2tp")
            te.transpose(w2tp[:], w2e[:], ident[:])
            w2T_bf = constp.tile([QT, 128], bf16, tag="w2T_bf")
            v.tensor_copy(w2T_bf[:], w2tp[:])
            w2b_ps = ps_t.tile([128, QT, 128], fp32, tag="tp", name="w2b_ps")
            for qt in range(QT):
                te.matmul(w2b_ps[:, qt, :],
                          sel_bf[0:QT, qt * 128:(qt + 1) * 128], w2T_bf[:],
                          start=True, stop=True)
            w2b = constp.tile([128, NQ], bf16, tag="w2b")
            sc.copy(w2b[:], w2b_ps[:].rearrange("p a q -> p (a q)"))

            # ---- L. AllGather 2 + panels (two halves for earlier start) ----
            if no_cc:
                dma.dma_start(
                    ag2_out.rearrange("(r q) c -> r q c", r=NC),
                    ag2_in.rearrange("(o q) c -> o q c", o=1).to_broadcast(
                        (NC, NQ, AGC2)))
            else:
                g.collective_compute(
                    "AllGather", OP.bypass,
                    ins=[ag2_in.opt()], outs=[ag2_out.opt()],
                    replica_groups=[list(range(NC))],
                )
            pan2 = constp.tile([128, JT, AGC2], fp32, tag="pan2")
            hx2 = constp.tile([128, JT, 17], bf16, tag="hx2")
            b2 = constp.tile([128, JT], fp32, tag="b2")
            d2 = constp.tile([128, JT], fp32, tag="d2")
            nb2 = constp.tile([128, JT], fp32, tag="nb2")
            b2_bf = constp.tile([128, JT], bf16, tag="b2_bf")
            HJ = JT // 2
            for hf in range(2):
                js = slice(hf * HJ, (hf + 1) * HJ)
                dma.dma_start(
                    pan2[:, js, :],
                    ag2_out[hf * (N // 2):(hf + 1) * (N // 2), :].rearrange(
                        "(t p) c -> p t c", p=128))
                sc.copy(hx2[:, js, :], pan2[:, js, 0:17])
                sc.activation(b2[:, js], pan2[:, js, 17], AF.Exp)
                sc.activation(d2[:, js], pan2[:, js, 17], AF.Exp, scale=ALPHA)
                v.tensor_scalar(nb2[:, js], b2[:, js], -1.0, None, OP.mult)
                v.tensor_copy(b2_bf[:, js], b2[:, js])

            # ---- M. layer-2 attention ----
            acc2 = ps_acc.tile([128, QT, 17], fp32, tag="acc", name="acc2")
            hb2_ps = ps_hb.tile([1, 17], fp32, tag="hb", name="hb2_ps")
            n_act2 = 0
            for jt in range(JT):
                e = _SCHED2[jt]
                pt = ppool.tile([128, NQ], bf16, tag="pt", name="pt2")
                if e == 'A':
                    sc.activation(pt[:], w2b[:], AF.Relu,
                                  bias=nb2[:, jt:jt + 1], scale=d2[:, jt:jt + 1])
                    te.matmul(hb2_ps[:], b2_bf[:, jt:jt + 1], hx2[:, jt, :],
                              start=(n_act2 == 0), stop=(jt == 27))
                    n_act2 += 1
                else:
                    eng = v if e == 'D' else g
                    eng.tensor_scalar(pt[:], w2b[:], d2[:, jt:jt + 1],
                                      b2[:, jt:jt + 1], OP.mult, OP.max)
                for qc in range(QT):
                    te.matmul(acc2[:, qc, :], pt[:, qc * 128:(qc + 1) * 128],
                              hx2[:, jt, :], start=(jt == 0), stop=False)
            hb2_sb = workp.tile([1, 17], bf16, tag="hb2_sb")
            v.tensor_copy(hb2_sb[:], hb2_ps[:])
            for qc in range(QT):
                te.matmul(acc2[:, qc, :], ones1[:], hb2_sb[:],
                          start=False, stop=True)

            # ---- N. normalize, elu, log_softmax, store ----
            r2 = workp.tile([128, QT], fp32, tag="r2")
            v.reciprocal(r2[:], acc2[:, :, 16])
            o_all = workp.tile([128, QT, 16], fp32, tag="o_all")
            for qc in range(QT):
                sc.activation(o_all[:, qc, :], acc2[:, qc, 0:16], AF.Copy,
                              scale=r2[:, qc:qc + 1])
            e2 = workp.tile([128, QT, 16], fp32, tag="e2")
            sc.activation(e2[:].rearrange("p a c -> p (a c)"),
                          o_all[:].rearrange("p a c -> p (a c)"), AF.Exp)
            v.tensor_scalar(e2[:].rearrange("p a c -> p (a c)"),
                            e2[:].rearrange("p a c -> p (a c)"),
                            1.0, 0.0, OP.subtract, OP.min)
            v.tensor_tensor(o_all[:].rearrange("p a c -> p (a c)"),
                            o_all[:].rearrange("p a c -> p (a c)"),
                            e2[:].rearrange("p a c -> p (a c)"), OP.max)
            ee = workp.tile([128, QT, 16], fp32, tag="ee")
            sc.activation(ee[:].rearrange("p a c -> p (a c)"),
                          o_all[:].rearrange("p a c -> p (a c)"), AF.Exp)
            s2s = workp.tile([128, QT], fp32, tag="s2s")
            v.tensor_reduce(s2s[:], ee[:], AX, OP.add)
            lse = workp.tile([128, QT], fp32, tag="lse")
            sc.activation(lse[:], s2s[:], AF.Ln)
            fin = workp.tile([128, QT, 16], fp32, tag="fin")
            for qc in range(QT):
                v.tensor_scalar(fin[:, qc, :], o_all[:, qc, :],
                                lse[:, qc:qc + 1], None, OP.subtract)
            dma.dma_start(out.rearrange("(a p) c -> p a c", p=128), fin[:])

    nc.finalize()
    return nc


def _get_compiled(no_cc=False, no_l1=False):
    key = ("nc", no_cc, no_l1)
    if key not in _CACHE:
        _CACHE[key] = _build_nc(no_cc=no_cc, no_l1=no_l1)
    return _CACHE[key]


def kernel(x, Wh, ah, Wo, ao):
    import ml_dtypes
    from concourse.bass_utils import run_bass_kernel_spmd

    bf = ml_dtypes.bfloat16
    nc = _get_compiled()
    x = np.asarray(x, np.float32)
    Wh = np.asarray(Wh, np.float32)
    ah = np.asarray(ah, np.float32)
    Wo = np.asarray(Wo, np.float32)
    ao = np.asarray(ao, np.float32)

    # host-side relayouts (no math): head-major weight matrix, its transpose,
    # block-diag score matrix, split ao
    Whr = np.ascontiguousarray(
        Wh.transpose(1, 0, 2).reshape(NFEAT, HW))          # [512, 512]
    WhrT = np.ascontiguousarray(Whr.T)
    Asd = np.zeros((HW, 16), np.float32)
    for h in range(NHEADS):
        Asd[h * NHID:(h + 1) * NHID, h] = ah[h, :NHID]      # src
        Asd[h * NHID:(h + 1) * NHID, 8 + h] = ah[h, NHID:]  # dst
    aod = np.stack([ao[:NCLASS], ao[NCLASS:]])              # [2, 16]

    Whr_b = Whr.astype(bf)
    WhrT_b = WhrT.astype(bf)
    Asd_b = Asd.astype(bf)
    Wo_b = np.ascontiguousarray(Wo).astype(bf)

    in_maps = []
    for i in range(NC):
        in_maps.append({
            "xT": np.ascontiguousarray(x[i * NQ:(i + 1) * NQ].T).astype(bf),
            "Whr": Whr_b, "WhrT": WhrT_b, "Asd": Asd_b,
            "Wo": Wo_b, "aod": aod,
        })
    res = run_bass_kernel_spmd(nc, in_maps, list(range(NC)))
    return np.concatenate([res.results[i]["out"] for i in range(NC)], 0)


# revision 25
# speedup vs baseline: 1.4930x; 1.0336x over previous
"""GAT (2-layer, 8-head) fused Bass kernel for 8 trn2 NeuronCores.

Sharding: nodes (rows of x) split 512/core. Per core: h computed key-major
with fused score columns; h (bf16) + s_dst (fp32) AllGather'd; each core
computes its 512xN attention block for all 8 heads; layer-1 output projected
and AllGather'd (18 fp32 cols); each core computes its 512xN layer-2 block
and the final log_softmax rows.

Key algebra: with s_i = h_i . a_src, d_j = h_j . a_dst,
  exp(leakyrelu(s_i + d_j)) = max(exp(s_i)exp(d_j), exp(.2 s_i)exp(.2 d_j))
and softmax over j is invariant to any per-i scale, so the attention
numerator is P[j,i] = max(b_j, w_i * dd_j) with b_j = exp(d_j),
w_i = exp(-0.8 s_i), dd_j = exp(0.2 d_j).

P tiles [128 keys, 512 queries] are produced on three engines:
  DVE/Pool: tensor_scalar (mult, max) -> P
  ACT:      relu(dd_j * w_i - b_j) = P - b_j, single activation op; the
            missing rank-1 term hb[c] = sum_j b_j hx[j,c] over ACT-tiles is
            added back into the PSUM accumulation via two tiny matmuls.
Attention matmuls run with the P chunk [128k x 128q] as the *stationary*
operand and the per-head hx block [128, 65] (64 h cols + ones) as the
*moving* operand: 65 columns/matmul instead of 512 -> ~2x less PE time,
and the output lands query-major so normalize/elu/log_softmax use cheap
per-partition scalars.
"""

import numpy as np

N, NFEAT, NHID, NCLASS, NHEADS = 4096, 512, 64, 16, 8
NC = 8                      # cores
NQ = N // NC                # 512 own nodes per core
QT = NQ // 128              # 4 query tiles per core
JT = N // 128               # 32 key tiles
NCH = JT // NC              # 4 key tiles per AG chunk
ALPHA = 0.2
HW = NHID * NHEADS          # 512
HXC = NHEADS * (NHID + 1)   # 520: per-head 64 h cols + ones col
AGC2 = 18                   # AG2: 16 outh + 1 ones + 1 sdst2

# engine schedule for the 32 P-tiles of each layer-1 head sweep
_ACT1 = (7, 14, 21, 28)
_SCHED1 = ['D'] * JT
for _p in _ACT1:
    _SCHED1[_p] = 'A'
for _p in (2, 9, 16, 23, 30):
    _SCHED1[_p] = 'P'
# layer-2: 32 tiles
_SCHED2 = ['D'] * JT
for _p in (3, 11, 19, 27):
    _SCHED2[_p] = 'A'
for _p in (6, 14, 22, 29):
    _SCHED2[_p] = 'P'

_CACHE = {}


def _build_nc(no_cc=False, no_l1=False):
    import concourse.bass as bass
    import concourse.bacc as bacc
    import concourse.mybir as mybir
    import concourse.tile as tile
    from concourse.masks import make_identity

    fp32 = mybir.dt.float32
    bf16 = mybir.dt.bfloat16
    AX = mybir.AxisListType.X
    OP = mybir.AluOpType
    AF = mybir.ActivationFunctionType

    nc = bacc.Bacc()
    xT = nc.declare_dram_parameter("xT", [NFEAT, NQ], bf16, isOutput=False)
    Whr = nc.declare_dram_parameter("Whr", [NFEAT, HW], bf16, isOutput=False)
    WhrT = nc.declare_dram_parameter("WhrT", [HW, NFEAT], bf16, isOutput=False)
    Asd = nc.declare_dram_parameter("Asd", [HW, 16], bf16, isOutput=False)
    Wo = nc.declare_dram_parameter("Wo", [HW, NCLASS], bf16, isOutput=False)
    aod = nc.declare_dram_parameter("aod", [2, NCLASS], fp32, isOutput=False)
    out = nc.declare_dram_parameter("out", [NQ, NCLASS], fp32, isOutput=True)

    with tile.TileContext(nc) as tc:
        with (
            tc.tile_pool(name="const", bufs=1) as constp,
            tc.tile_pool(name="big", bufs=1) as bigp,
            tc.tile_pool(name="work", bufs=3) as workp,
            tc.tile_pool(name="pp", bufs=44) as ppool,
            tc.tile_pool(name="ps_acc", bufs=3, space="PSUM") as ps_acc,
            tc.tile_pool(name="ps_t", bufs=3, space="PSUM") as ps_t,
            tc.tile_pool(name="ps_hb", bufs=2, space="PSUM") as ps_hb,
            tc.tile_pool(name="dram", bufs=1, space="DRAM") as dramp,
        ):
            v, sc, g, te, dma = nc.vector, nc.scalar, nc.gpsimd, nc.tensor, nc.sync

            ident = constp.tile([128, 128], fp32, tag="ident")
            make_identity(nc, ident[:])
            ident_bf = constp.tile([128, 128], bf16, tag="ident_bf")
            v.tensor_copy(ident_bf[:], ident[:])
            ones1 = constp.tile([1, 128], bf16, tag="ones1")
            g.memset(ones1[:], 1.0)
            # sel[k, h*128+m] = 1 iff k == h (partition-broadcast matmuls)
            self_f = constp.tile([8, 8 * 128], fp32, tag="self_f")
            g.memset(self_f[:], 0.0)
            g.affine_select(
                out=self_f[:].rearrange("k (h m) -> k h m", m=128),
                in_=self_f[:].rearrange("k (h m) -> k h m", m=128),
                compare_op=mybir.AluOpType.not_equal,
                fill=1.0, base=0, channel_multiplier=1,
                pattern=[[-1, 8], [0, 128]])
            sel_bf = constp.tile([8, 8 * 128], bf16, tag="sel_bf")
            sc.copy(sel_bf[:], self_f[:])

            # ---- A. param loads (WhrT/Asd first: the score chain gates the
            # attention panels, which gate all P-tile production) ----
            whrT_sb = constp.tile([128, 4, NFEAT], bf16, tag="whrT_sb")
            dma.dma_start(whrT_sb[:], WhrT.rearrange("(k p) f -> p k f", p=128))
            asd_sb = constp.tile([128, 4, 16], bf16, tag="asd_sb")
            dma.dma_start(asd_sb[:], Asd.rearrange("(k p) s -> p k s", p=128))
            xT_sb = constp.tile([128, 4, NQ], bf16, tag="xT_sb")
            dma.dma_start(xT_sb[:], xT.rearrange("(k p) q -> p k q", p=128))
            whr_sb = constp.tile([128, 4, HW], bf16, tag="whr_sb")
            dma.dma_start(whr_sb[:], Whr.rearrange("(k p) c -> p k c", p=128))
            wo_sb = constp.tile([128, 4, 16], bf16, tag="wo_sb")
            dma.dma_start(wo_sb[:], Wo.rearrange("(k p) s -> p k s", p=128))
            aos_b = constp.tile([128, 16], fp32, tag="aos_b")
            dma.dma_start(aos_b[:], aod[0:1, :].to_broadcast((128, 16)))
            aod_b = constp.tile([128, 16], fp32, tag="aod_b")
            dma.dma_start(aod_b[:], aod[1:2, :].to_broadcast((128, 16)))

            ag1h_in = dramp.tile([NQ, HXC], bf16, tag="ag1h_in")
            ag1h_out = dramp.tile([N, HXC], bf16, tag="ag1h_out",
                                  addr_space="Local" if no_cc else "Shared")
            ag1s_in = dramp.tile([NQ, 8], fp32, tag="ag1s_in")
            ag1s_out = dramp.tile([N, 8], fp32, tag="ag1s_out",
                                  addr_space="Local" if no_cc else "Shared")
            ag2_in = dramp.tile([NQ, AGC2], fp32, tag="ag2_in")
            ag2_out = dramp.tile([N, AGC2], fp32, tag="ag2_out",
                                 addr_space="Local" if no_cc else "Shared")

            # ---- C. Wa_feat = Whr @ Asd; s_own; stage + AllGather s ----
            wa_ps = ps_t.tile([16, NFEAT], fp32, tag="tp", name="wa_ps")
            for k in range(4):
                te.matmul(wa_ps[:], asd_sb[:, k, :], whrT_sb[:, k, :],
                          start=(k == 0), stop=(k == 3))
            waT_sb = constp.tile([16, NFEAT], bf16, tag="waT_sb")
            sc.copy(waT_sb[:], wa_ps[:])
            waf_ps = ps_t.tile([128, 4, 16], bf16, tag="tp", name="waf_ps")
            for k in range(4):
                te.transpose(waf_ps[:, k, :], waT_sb[:, k * 128:(k + 1) * 128],
                             ident_bf[0:16, 0:16])
            wa_f = constp.tile([128, 4, 16], bf16, tag="wa_f")
            v.tensor_copy(wa_f[:], waf_ps[:])

            stgs = [bigp.tile([128, 16], fp32, tag=f"stgs{qt}",
                              name=f"stgs{qt}") for qt in range(QT)]
            for qt in range(QT):
                s_ps = ps_t.tile([128, 16], fp32, tag="tp", name="s_ps")
                for k in range(4):
                    te.matmul(s_ps[:], xT_sb[:, k, qt * 128:(qt + 1) * 128],
                              wa_f[:, k, :], start=(k == 0), stop=(k == 3))
                v.tensor_copy(stgs[qt][:], s_ps[:])
                dma.dma_start(ag1s_in[qt * 128:(qt + 1) * 128, :],
                              stgs[qt][:, 8:16])
            if no_cc:
                dma.dma_start(
                    ag1s_out.rearrange("(r q) h -> r q h", r=NC),
                    ag1s_in.rearrange("(o q) h -> o q h", o=1).to_broadcast(
                        (NC, NQ, 8)))
            else:
                g.collective_compute(
                    "AllGather", OP.bypass,
                    ins=[ag1s_in.opt()], outs=[ag1s_out.opt()],
                    replica_groups=[list(range(NC))],
                )
            sd_pan = constp.tile([128, JT * 8], fp32, tag="sd_pan")
            dma.dma_start(
                sd_pan[:].rearrange("p (t h) -> p t h", h=8),
                ag1s_out.rearrange("(t p) h -> p t h", p=128))

            # ---- D. w panel (own s_src): transpose, exp, broadcast ----
            s_fm = ps_t.tile([16, NQ], fp32, tag="tp", name="s_fm")
            for qt in range(QT):
                te.transpose(s_fm[:, qt * 128:(qt + 1) * 128], stgs[qt][:],
                             ident[0:128, 0:128])
            w_bf = constp.tile([8, NQ], bf16, tag="w_bf")
            sc.activation(w_bf[:], s_fm[0:8, :], AF.Exp, scale=-0.8)
            wb_all = constp.tile([128, NHEADS, NQ], bf16, tag="wb_all")
            for h in range(NHEADS):
                wb_ps = ps_t.tile([128, NQ], fp32, tag="tp", name="wb_ps")
                te.matmul(wb_ps[:], sel_bf[:, h * 128:(h + 1) * 128], w_bf[:],
                          start=True, stop=True)
                sc.copy(wb_all[:, h, :], wb_ps[:])

            # ---- B. h_own key-major; stage + AllGather h ----
            stg = [bigp.tile([128, HXC], bf16, tag=f"stg{qt}", name=f"stg{qt}")
                   for qt in range(QT)]
            for qt in range(QT):
                h_ps = ps_acc.tile([128, HW], fp32, tag="acc", name="h_ps")
                for k in range(4):
                    te.matmul(h_ps[:], xT_sb[:, k, qt * 128:(qt + 1) * 128],
                              whr_sb[:, k, :], start=(k == 0), stop=(k == 3))
                eng = sc if qt % 2 else v
                eng_c = eng.copy if qt % 2 else eng.tensor_copy
                eng_c(stg[qt][:].rearrange("p (h c) -> p h c", c=65)[:, :, 0:64],
                      h_ps[:].rearrange("p (h c) -> p h c", c=64))
                g.memset(
                    stg[qt][:].rearrange("p (h c) -> p h c", c=65)[:, :, 64:65],
                    1.0)
                dma.dma_start(ag1h_in[qt * 128:(qt + 1) * 128, :], stg[qt][:])

            # ---- E. AG1h replica copies interleaved with hx chunk loads ----
            hx = []
            for c in range(NC):
                if no_cc:
                    dma.dma_start(ag1h_out[c * NQ:(c + 1) * NQ, :],
                                  ag1h_in[:])
                elif c == 0:
                    g.collective_compute(
                        "AllGather", OP.bypass,
                        ins=[ag1h_in.opt()], outs=[ag1h_out.opt()],
                        replica_groups=[list(range(NC))],
                    )
                t = bigp.tile([128, NCH, HXC], bf16, tag=f"hx{c}",
                              name=f"hx{c}")
                dma.dma_start(
                    t[:],
                    ag1h_out[c * NQ:(c + 1) * NQ, :].rearrange(
                        "(a p) x -> p a x", p=128))
                hx.append(t)

            # ---- F. key-side score panels ----
            b_all = constp.tile([128, JT * 8], fp32, tag="b_all")
            sc.activation(b_all[:], sd_pan[:], AF.Exp)
            d_all = constp.tile([128, JT * 8], fp32, tag="d_all")
            sc.activation(d_all[:], sd_pan[:], AF.Exp, scale=ALPHA)
            nb_all = constp.tile([128, JT * 8], fp32, tag="nb_all")
            v.tensor_scalar(nb_all[:], b_all[:], -1.0, None, OP.mult)
            b_bf = constp.tile([128, JT * 8], bf16, tag="b_bf")
            v.tensor_copy(b_bf[:], b_all[:])

            def hx_slice(jt, h, w):
                return hx[jt // NCH][:, jt % NCH, h * 65:h * 65 + w]

            # ---- H. layer-1 attention ----
            xc_pre = [bigp.tile([128, HW], fp32, tag=f"xc{qc}",
                                name=f"xc{qc}") for qc in range(QT)]
            r_pan = constp.tile([128, NHEADS, QT], fp32, tag="r_pan")

            for h in range(NHEADS if not no_l1 else 0):
                acc = ps_acc.tile([128, QT, 65], fp32, tag="acc", name="acc")
                act_jts = []
                hb_ps = ps_hb.tile([1, 65], fp32, tag="hb", name="hb_ps")
                for idx in range(JT):
                    jt = (h * NCH + idx) % JT
                    col = slice(jt * 8 + h, jt * 8 + h + 1)
                    e = _SCHED1[idx]
                    pt = ppool.tile([128, NQ], bf16, tag="pt", name="pt")
                    if e == 'A':
                        sc.activation(pt[:], wb_all[:, h, :], AF.Relu,
                                      bias=nb_all[:, col], scale=d_all[:, col])
                        te.matmul(hb_ps[:], b_bf[:, col], hx_slice(jt, h, 65),
                                  start=(not act_jts), stop=(idx == _ACT1[-1]))
                        act_jts.append(jt)
                    else:
                        eng = v if e == 'D' else g
                        eng.tensor_scalar(pt[:], wb_all[:, h, :],
                                          d_all[:, col], b_all[:, col],
                                          OP.mult, OP.max)
                    for qc in range(QT):
                        te.matmul(acc[:, qc, :],
                                  pt[:, qc * 128:(qc + 1) * 128],
                                  hx_slice(jt, h, 65),
                                  start=(idx == 0), stop=False)
                # rank-1 correction for the ACT-produced tiles: acc += 1 (x) hb
                hb_sb = workp.tile([1, 65], bf16, tag="hb_sb", bufs=2)
                v.tensor_copy(hb_sb[:], hb_ps[:])
                for qc in range(QT):
                    te.matmul(acc[:, qc, :], ones1[:], hb_sb[:],
                              start=False, stop=True)
                # normalize: r = 1/den, xc_pre[:, h*64:] = f * r
                v.reciprocal(r_pan[:, h, :], acc[:, :, 64])
                for qc in range(QT):
                    sc.activation(xc_pre[qc][:, h * 64:(h + 1) * 64],
                                  acc[:, qc, 0:64], AF.Copy,
                                  scale=r_pan[:, h, qc:qc + 1])

            if no_l1:
                for qc in range(QT):
                    g.memset(xc_pre[qc][:], 0.5)

            # ---- I/J. per-query-tile: elu (fp32), transpose, project,
            # stage AG2 ----
            w2tmp = constp.tile([128, QT], fp32, tag="w2tmp")
            for qc in range(QT):
                e1 = workp.tile([128, HW], fp32, tag="elu_e", name="e1")
                sc.activation(e1[:], xc_pre[qc][:], AF.Exp)
                g.tensor_scalar(e1[:], e1[:], 1.0, 0.0, OP.subtract, OP.min)
                g.scalar_tensor_tensor(xc_pre[qc][:], e1[:], 0.0,
                                       xc_pre[qc][:], OP.add, OP.max)
                tp = ps_t.tile([128, 4, 128], fp32, tag="tp", name="tp_xc")
                for fc in range(4):
                    te.transpose(tp[:, fc, :],
                                 xc_pre[qc][:, fc * 128:(fc + 1) * 128],
                                 ident[:])
                xcT = bigp.tile([128, 4, 128], bf16, tag=f"xcT{qc}",
                                name=f"xcT{qc}")
                sc.copy(xcT[:], tp[:])
                o_ps = ps_t.tile([128, 16], fp32, tag="tp", name="o_ps")
                for fc in range(4):
                    te.matmul(o_ps[:], xcT[:, fc, :], wo_sb[:, fc, :],
                              start=(fc == 0), stop=(fc == 3))
                stg2 = bigp.tile([128, AGC2], fp32, tag=f"stg2_{qc}",
                                 name=f"stg2_{qc}")
                v.tensor_copy(stg2[:, 0:16], o_ps[:])
                g.memset(stg2[:, 16:17], 1.0)
                tmp = workp.tile([128, 16], fp32, tag="sdtmp")
                v.scalar_tensor_tensor(tmp[:], o_ps[:], 1.0, aod_b[:],
                                       OP.mult, OP.mult,
                                       accum_out=stg2[:, 17:18])
                tmp2 = workp.tile([128, 16], fp32, tag="sdtmp2")
                v.scalar_tensor_tensor(tmp2[:], o_ps[:], 1.0, aos_b[:],
                                       OP.mult, OP.mult,
                                       accum_out=w2tmp[:, qc:qc + 1])
                dma.dma_start(ag2_in[qc * 128:(qc + 1) * 128, :], stg2[:])

            # ---- K. w2 broadcast panel ----
            w2e = constp.tile([128, QT], fp32, tag="w2e")
            sc.activation(w2e[:], w2tmp[:], AF.Exp, scale=-0.8)
            w2tp = ps_hb.tile([QT, 128], fp32, tag="hb", name="w2tp")
            te.transpose(w2tp[:], w2e[:], ident[:])
            w2T_bf = constp.tile([QT, 128], bf16, tag="w2T_bf")
            v.tensor_copy(w2T_bf[:], w2tp[:])
            w2b_ps = ps_t.tile([128, QT, 128], fp32, tag="tp", name="w2b_ps")
            for qt in range(QT):
                te.matmul(w2b_ps[:, qt, :],
                          sel_bf[0:QT, qt * 128:(qt + 1) * 128], w2T_bf[:],
                          start=True, stop=True)
            w2b = constp.tile([128, NQ], bf16, tag="w2b")
            sc.copy(w2b[:], w2b_ps[:].rearrange("p a q -> p (a q)"))

            # ---- L. AllGather 2 + panels (two halves for earlier start) ----
            if no_cc:
                dma.dma_start(
                    ag2_out.rearrange("(r q) c -> r q c", r=NC),
                    ag2_in.rearrange("(o q) c -> o q c", o=1).to_broadcast(
                        (NC, NQ, AGC2)))
            else:
                g.collective_compute(
                    "AllGather", OP.bypass,
                    ins=[ag2_in.opt()], outs=[ag2_out.opt()],
                    replica_groups=[list(range(NC))],
                )
            pan2 = constp.tile([128, JT, AGC2], fp32, tag="pan2")
            hx2 = constp.tile([128, JT, 17], bf16, tag="hx2")
            b2 = constp.tile([128, JT], fp32, tag="b2")
            d2 = constp.tile([128, JT], fp32, tag="d2")
            nb2 = constp.tile([128, JT], fp32, tag="nb2")
            b2_bf = constp.tile([128, JT], bf16, tag="b2_bf")
            HJ = JT // 2
            for hf in range(2):
                js = slice(hf * HJ, (hf + 1) * HJ)
                dma.dma_start(
                    pan2[:, js, :],
                    ag2_out[hf * (N // 2):(hf + 1) * (N // 2), :].rearrange(
                        "(t p) c -> p t c", p=128))
                sc.copy(hx2[:, js, :], pan2[:, js, 0:17])
                sc.activation(b2[:, js], pan2[:, js, 17], AF.Exp)
                sc.activation(d2[:, js], pan2[:, js, 17], AF.Exp, scale=ALPHA)
                v.tensor_scalar(nb2[:, js], b2[:, js], -1.0, None, OP.mult)
                v.tensor_copy(b2_bf[:, js], b2[:, js])

            # ---- M. layer-2 attention ----
            acc2 = ps_acc.tile([128, QT, 17], fp32, tag="acc", name="acc2")
            hb2_ps = ps_hb.tile([1, 17], fp32, tag="hb", name="hb2_ps")
            n_act2 = 0
            for jt in range(JT):
                e = _SCHED2[jt]
                pt = ppool.tile([128, NQ], bf16, tag="pt", name="pt2")
                if e == 'A':
                    sc.activation(pt[:], w2b[:], AF.Relu,
                                  bias=nb2[:, jt:jt + 1], scale=d2[:, jt:jt + 1])
                    te.matmul(hb2_ps[:], b2_bf[:, jt:jt + 1], hx2[:, jt, :],
                              start=(n_act2 == 0), stop=(jt == 27))
                    n_act2 += 1
                else:
                    eng = v if e == 'D' else g
                    eng.tensor_scalar(pt[:], w2b[:], d2[:, jt:jt + 1],
                                      b2[:, jt:jt + 1], OP.mult, OP.max)
                for qc in range(QT):
                    te.matmul(acc2[:, qc, :], pt[:, qc * 128:(qc + 1) * 128],
                              hx2[:, jt, :], start=(jt == 0), stop=False)
            hb2_sb = workp.tile([1, 17], bf16, tag="hb2_sb")
            v.tensor_copy(hb2_sb[:], hb2_ps[:])
            for qc in range(QT):
                te.matmul(acc2[:, qc, :], ones1[:], hb2_sb[:],
                          start=False, stop=True)

            # ---- N. normalize, elu, log_softmax, store ----
            r2 = workp.tile([128, QT], fp32, tag="r2")
            v.reciprocal(r2[:], acc2[:, :, 16])
            o_all = workp.tile([128, QT, 16], fp32, tag="o_all")
            for qc in range(QT):
                sc.activation(o_all[:, qc, :], acc2[:, qc, 0:16], AF.Copy,
                              scale=r2[:, qc:qc + 1])
            e2 = workp.tile([128, QT, 16], fp32, tag="e2")
            sc.activation(e2[:].rearrange("p a c -> p (a c)"),
                          o_all[:].rearrange("p a c -> p (a c)"), AF.Exp)
            v.tensor_scalar(e2[:].rearrange("p a c -> p (a c)"),
                            e2[:].rearrange("p a c -> p (a c)"),
                            1.0, 0.0, OP.subtract, OP.min)
            v.tensor_tensor(o_all[:].rearrange("p a c -> p (a c)"),
                            o_all[:].rearrange("p a c -> p (a c)"),
                            e2[:].rearrange("p a c -> p (a c)"), OP.max)
            ee = workp.tile([128, QT, 16], fp32, tag="ee")
            sc.activation(ee[:].rearrange("p a c -> p (a c)"),
                          o_all[:].rearrange("p a c -> p (a c)"), AF.Exp)
            s2s = workp.tile([128, QT], fp32, tag="s2s")
            v.tensor_reduce(s2s[:], ee[:], AX, OP.add)
            lse = workp.tile([128, QT], fp32, tag="lse")
            sc.activation(lse[:], s2s[:], AF.Ln)
            fin = workp.tile([128, QT, 16], fp32, tag="fin")
            for qc in range(QT):
                v.tensor_scalar(fin[:, qc, :], o_all[:, qc, :],
                                lse[:, qc:qc + 1], None, OP.subtract)
            dma.dma_start(out.rearrange("(a p) c -> p a c", p=128), fin[:])

    nc.finalize()
    return nc


def _get_compiled(no_cc=False, no_l1=False):
    key = ("nc", no_cc, no_l1)
    if key not in _CACHE:
        _CACHE[key] = _build_nc(no_cc=no_cc, no_l1=no_l1)
    return _CACHE[key]


def kernel(x, Wh, ah, Wo, ao):
    import ml_dtypes
    from concourse.bass_utils import run_bass_kernel_spmd

    bf = ml_dtypes.bfloat16
    nc = _get_compiled()
    x = np.asarray(x, np.float32)
    Wh = np.asarray(Wh, np.float32)
    ah = np.asarray(ah, np.float32)
    Wo = np.asarray(Wo, np.float32)
    ao = np.asarray(ao, np.float32)

    # host-side relayouts (no math): head-major weight matrix, its transpose,
    # block-diag score matrix, split ao
    Whr = np.ascontiguousarray(
        Wh.transpose(1, 0, 2).reshape(NFEAT, HW))          # [512, 512]
    WhrT = np.ascontiguousarray(Whr.T)
    Asd = np.zeros((HW, 16), np.float32)
    for h in range(NHEADS):
        Asd[h * NHID:(h + 1) * NHID, h] = ah[h, :NHID]      # src
        Asd[h * NHID:(h + 1) * NHID, 8 + h] = ah[h, NHID:]  # dst
    aod = np.stack([ao[:NCLASS], ao[NCLASS:]])              # [2, 16]

    Whr_b = Whr.astype(bf)
    WhrT_b = WhrT.astype(bf)
    Asd_b = Asd.astype(bf)
    Wo_b = np.ascontiguousarray(Wo).astype(bf)

    in_maps = []
    for i in range(NC):
        in_maps.append({
            "xT": np.ascontiguousarray(x[i * NQ:(i + 1) * NQ].T).astype(bf),
            "Whr": Whr_b, "WhrT": WhrT_b, "Asd": Asd_b,
            "Wo": Wo_b, "aod": aod,
        })
    res = run_bass_kernel_spmd(nc, in_maps, list(range(NC)))
    return np.concatenate([res.results[i]["out"] for i in range(NC)], 0)
